# revision 8
# baseline (speedup 1.0000x reference)
"""3-layer GraphSAGE (mean agg) on 8 Trainium2 NeuronCores — two-phase radix.

The old per-edge dma_gather spent ~9ns/edge of GPSIMD(Q7) descriptor
generation (3.1ms of a 3.6ms kernel).  Measured on HW: descriptor cost is
per-INDEX, independent of element size.  So:
  Phase 1 (expand): stream the replicated node table sequentially; for each
    128-row table window, a PE one-hot matmul scatters its outgoing edge rows
    into a dst-ordered slot array (8-slot runs, each run pure to one
    dst-window-pair); slots stream to a DRAM staging buffer via large DMAs.
  Phase 2 (collect): per dst-window-pair, dma_gather with elem_size=2KB pulls
    8 edge-rows per descriptor (8x fewer Q7 descriptors); PE aggregates via
    8-position one-hot matmuls into per-window PSUM tiles; then the dense
    SAGE matmuls / ReLU / log_softmax tail as before.
Weights replicated; node features exchanged between layers via AllGather.
"""

import sys
import numpy as np

for _p in ("/opt/trn_rl_repo", "/root/.axon_site/_ro/trn_rl_repo"):
    if _p not in sys.path:
        sys.path.append(_p)

import ml_dtypes

BF16 = ml_dtypes.bfloat16

N = 50000
E = 800000
D_IN = 128
D_HID = 128
D_OUT = 64
N_CORES = 8
WIN = 128
NPC_RAW = N // N_CORES            # 6250
NPC = ((NPC_RAW + WIN - 1) // WIN) * WIN   # 6272
W = NPC // WIN                    # 49
NG = (W + 1) // 2                 # 25 dst-window groups (pairs, last single)
HTAB = N_CORES * NPC              # 50176
NWS = HTAB // WIN                 # 392 table windows
R = 8                             # slots per run (2KB bf16)
CHW = 32                          # table windows per phase-1 chunk
CHB = 16                          # blocks per staging chunk
MMC = 32                          # matmuls per P-strip build


def _plan(edge_index):
    src = np.asarray(edge_index[0], dtype=np.int64)
    dst = np.asarray(edge_index[1], dtype=np.int64)
    ec = dst // NPC_RAW
    rank = dst % NPC_RAW
    ew = rank // WIN
    dl = rank % WIN
    grp = ew // 2
    h = ew % 2
    ep = (src // NPC_RAW) * NPC + (src % NPC_RAW)
    ws = ep // WIN
    sl = ep % WIN

    cnt = np.zeros((N_CORES, NWS, NG), np.int64)
    np.add.at(cnt, (ec, ws, grp), 1)
    runs_cell = -(-cnt // R)                      # [8, NWS, NG]
    # slots per (core, ws): run-aligned
    s_c_ws = runs_cell.sum(axis=2) * R            # [8, NWS]
    REG = s_c_ws.max(axis=0)                      # [NWS] (multiple of 8)
    RB = np.concatenate([[0], np.cumsum(REG)])    # region base slots
    LSLOT = int(RB[-1])
    NBLK = -(-LSLOT // 128)
    LSLOT_PAD = NBLK * 128
    NRUN = LSLOT_PAD // R
    assert NRUN < 32768, f"run index overflow: {NRUN}"

    # per-core cell base slot
    cum_runs = np.cumsum(runs_cell, axis=2) - runs_cell    # exclusive, [8,NWS,NG]
    cb = RB[None, :NWS, None] + cum_runs * R               # [8, NWS, NG]

    # edge slot assignment
    okey = np.lexsort((dl, grp, ws, ec))
    ec_s, ws_s, grp_s, sl_s, dl_s, h_s = (a[okey] for a in (ec, ws, grp, sl, dl, h))
    cell_id = (ec_s * NWS + ws_s) * NG + grp_s
    newcell = np.concatenate([[True], cell_id[1:] != cell_id[:-1]])
    gstart = np.maximum.accumulate(np.where(newcell, np.arange(E), 0))
    krank = np.arange(E) - gstart
    slot = cb[ec_s, ws_s, grp_s] + krank

    srcl = np.full((N_CORES, LSLOT_PAD), -1.0, np.float32)
    dA = np.full((N_CORES, LSLOT_PAD), -1.0, np.float32)
    srcl[ec_s, slot] = sl_s
    dA[ec_s, slot] = dl_s + 128.0 * h_s

    # matmul descriptor list: blocks split into region segments
    reg_end = RB[1:]
    mm_block, mm_ws, mm_start, mm_stop, mm_lo, mm_hi = [], [], [], [], [], []
    for b in range(NBLK):
        lo, hi = b * 128, (b + 1) * 128
        wlo = int(np.searchsorted(reg_end, lo, side="right"))
        segs = []
        cur = lo
        wcur = min(wlo, NWS - 1)
        while cur < hi:
            seg_end = min(hi, int(reg_end[wcur]) if wcur < NWS else hi)
            if seg_end <= cur:       # dead tail past LSLOT
                seg_end = hi
            segs.append((wcur, cur, seg_end))
            cur = seg_end
            wcur = min(wcur + 1, NWS - 1)
        for i, (wsx, lo2, hi2) in enumerate(segs):
            mm_block.append(b)
            mm_ws.append(wsx)
            mm_start.append(i == 0)
            mm_stop.append(i == len(segs) - 1)
            mm_lo.append(lo2 - lo)
            mm_hi.append(hi2 - lo)
    n_mm = len(mm_block)

    PV = np.full((N_CORES, n_mm * 128), -1.0, np.float32)
    for m in range(n_mm):
        b, lo2, hi2 = mm_block[m], mm_lo[m], mm_hi[m]
        PV[:, m * 128 + lo2: m * 128 + hi2] = \
            srcl[:, b * 128 + lo2: b * 128 + hi2]

    # phase-2 run lists per group
    NRg, nrt, idx_w, VA = [], [], [], []
    for g in range(NG):
        rls = []
        for c in range(N_CORES):
            parts = []
            for wsx in range(NWS):
                k = int(runs_cell[c, wsx, g])
                if k:
                    base = int(cb[c, wsx, g]) // R
                    parts.append(base + np.arange(k))
            rl = np.concatenate(parts) if parts else np.zeros(0, np.int64)
            rls.append(rl)
        mx = max(len(r) for r in rls)
        nr = ((mx + 127) // 128) * 128
        NRg.append(nr)
        nrt.append(nr // 128)
        iw, va = [], []
        for c in range(N_CORES):
            rl = np.zeros(nr, np.int64)
            rl[: len(rls[c])] = rls[c]
            # wrap for dma_gather: per 512-call columns of reshape(-1,16).T
            cols = []
            for b0 in range(0, nr, 512):
                blk = rl[b0: b0 + 512]
                cols.append(blk.reshape(-1, 16).T)
            iw.append(np.concatenate(cols, axis=1).astype(np.int16))
            a = dA[c][(rl[:, None] * R + np.arange(R))].astype(np.float32)
            if len(rls[c]) < nr:    # padded runs contribute nothing
                a[len(rls[c]):] = -1.0
            va.append(a.reshape(nr // 128, 128, R).transpose(1, 0, 2)
                      .reshape(128, -1).astype(BF16))
        idx_w.append(iw)
        VA.append(va)

    deg = np.bincount(dst, minlength=N).astype(np.float32)
    invdeg = []
    for c in range(N_CORES):
        v = np.ones(NPC, np.float32)
        v[:NPC_RAW] = 1.0 / np.maximum(deg[c * NPC_RAW:(c + 1) * NPC_RAW], 1.0)
        invdeg.append(np.tile(v[None, :], (WIN, 1)))

    return {
        "LSLOT_PAD": LSLOT_PAD, "NBLK": NBLK, "NRUN": NRUN, "n_mm": n_mm,
        "mm_block": mm_block, "mm_ws": mm_ws, "mm_start": mm_start,
        "mm_stop": mm_stop, "NRg": NRg, "nrt": nrt,
        "PV": [np.ascontiguousarray(PV[c]).astype(BF16)[None, :] for c in range(N_CORES)],
        "idx_w": idx_w, "VA": VA, "invdeg": invdeg,
    }


def _build(plan):
    import concourse.bacc as bacc
    import concourse.bass as bass
    import concourse.mybir as mybir
    import concourse.tile as tile
    from contextlib import ExitStack

    dt = mybir.dt
    NBLK, NRUN, n_mm = plan["NBLK"], plan["NRUN"], plan["n_mm"]
    NRg, nrt = plan["NRg"], plan["nrt"]
    mm_block, mm_ws = plan["mm_block"], plan["mm_ws"]
    mm_start, mm_stop = plan["mm_start"], plan["mm_stop"]
    SNRT = sum(nrt)

    nc = bacc.Bacc("TRN2", target_bir_lowering=False)

    x_sl = nc.dram_tensor("x_slice", [NPC, D_IN], dt.float32, kind="ExternalInput")
    pv_d = nc.dram_tensor("pv", [1, n_mm * 128], dt.bfloat16, kind="ExternalInput")
    idx_d = nc.dram_tensor("idx", [128, (sum(NRg)) // 16], dt.int16,
                           kind="ExternalInput")
    va_d = nc.dram_tensor("va", [128, SNRT * R], dt.bfloat16, kind="ExternalInput")
    invdeg_d = nc.dram_tensor("invdeg", [128, NPC], dt.float32, kind="ExternalInput")
    idbf_d = nc.dram_tensor("idbf", [128, 128], dt.bfloat16, kind="ExternalInput")
    ioct_d = nc.dram_tensor("ioct", [128, MMC * 128], dt.bfloat16,
                            kind="ExternalInput")
    io2t_d = nc.dram_tensor("io2t", [128, R * 256], dt.bfloat16,
                            kind="ExternalInput")
    wl_d, wr_d, b_d = [], [], []
    for li, (din, dout) in enumerate([(D_IN, D_HID), (D_HID, D_HID),
                                      (D_HID, D_OUT)]):
        wl_d.append(nc.dram_tensor(f"wl{li}", [din, dout], dt.bfloat16,
                                   kind="ExternalInput"))
        wr_d.append(nc.dram_tensor(f"wr{li}", [din, dout], dt.bfloat16,
                                   kind="ExternalInput"))
        b_d.append(nc.dram_tensor(f"b{li}", [128, 1], dt.float32,
                                  kind="ExternalInput"))

    xsb = nc.dram_tensor("xsb", [NPC, D_IN], dt.bfloat16)
    hsl = [nc.dram_tensor(f"hsl{i}", [NPC, D_IN], dt.bfloat16) for i in range(2)]
    hf = [nc.dram_tensor(f"hf{i}", [HTAB, D_IN], dt.bfloat16) for i in range(3)]
    stag2 = [nc.dram_tensor(f"stag{i}", [NRUN, R * 128], dt.bfloat16)
             for i in range(2)]
    out_d = nc.dram_tensor("out", [NPC, D_OUT], dt.float32, kind="ExternalOutput")

    groups = [list(range(N_CORES))]

    with tile.TileContext(nc) as tc, ExitStack() as ctx:
        per = ctx.enter_context(tc.tile_pool(name="persist", bufs=1))
        xpool = ctx.enter_context(tc.tile_pool(name="xch", bufs=2))
        ppool = ctx.enter_context(tc.tile_pool(name="pstrip", bufs=2))
        cpool = ctx.enter_context(tc.tile_pool(name="chunk", bufs=3))
        gpool = ctx.enter_context(tc.tile_pool(name="g2", bufs=2))
        spool = ctx.enter_context(tc.tile_pool(name="sstrip", bufs=2))
        mpool = ctx.enter_context(tc.tile_pool(name="misc", bufs=2))
        pp_e = ctx.enter_context(tc.tile_pool(name="ps_exp", bufs=2, space="PSUM"))
        pp = ctx.enter_context(tc.tile_pool(name="ps_agg", bufs=2, space="PSUM"))
        pp_d = ctx.enter_context(tc.tile_pool(name="ps_dense", bufs=2, space="PSUM"))
        pp_t = ctx.enter_context(tc.tile_pool(name="ps_tr", bufs=1, space="PSUM"))

        def load_const(dram, shape, dtp, tag):
            t = per.tile(shape, dtp, tag=tag, name=tag)
            nc.sync.dma_start(out=t[:], in_=dram[:])
            return t

        idbf = load_const(idbf_d, [128, 128], dt.bfloat16, "idbf")
        ioct = load_const(ioct_d, [128, MMC * 128], dt.bfloat16, "ioct")
        io2t = load_const(io2t_d, [128, R * 256], dt.bfloat16, "io2t")
        invdeg = load_const(invdeg_d, [128, NPC], dt.float32, "invdeg")
        wl = [load_const(wl_d[i], list(wl_d[i].shape), dt.bfloat16, f"wl{i}")
              for i in range(3)]
        wr = [load_const(wr_d[i], list(wr_d[i].shape), dt.bfloat16, f"wr{i}")
              for i in range(3)]
        bias = [load_const(b_d[i], [128, 1], dt.float32, f"b{i}") for i in range(3)]
        idx_sb = load_const(idx_d, [128, sum(NRg) // 16], dt.int16, "idx")
        va_sb = load_const(va_d, [128, SNRT * R], dt.bfloat16, "va")

        # iotas
        iota_col = per.tile([128, 1], dt.bfloat16, tag="iotac")
        nc.gpsimd.iota(iota_col[:], pattern=[[1, 1]], base=0,
                       channel_multiplier=1, allow_small_or_imprecise_dtypes=True)
        iota_row = per.tile([128, 128], dt.bfloat16, tag="iotar")
        nc.gpsimd.iota(iota_row[:], pattern=[[1, 128]], base=0,
                       channel_multiplier=0, allow_small_or_imprecise_dtypes=True)
        iota2t_d = None
        zbias = per.tile([128, 1], dt.float32, tag="zbias")
        nc.gpsimd.memset(zbias[:], 0.0)

        idf = per.tile([128, 128], dt.float32, tag="idf")
        nc.vector.tensor_copy(out=idf[:], in_=idbf[:])
        hT = [per.tile([128, NPC], dt.bfloat16, tag=f"hT{i}", name=f"hT{i}")
              for i in range(2)]

        # bootstrap: bf16 cast + AllGather + hT[0]
        nc.gpsimd.dma_start(out=xsb[:, :], in_=x_sl[:, :])
        nc.gpsimd.collective_compute(
            "AllGather", mybir.AluOpType.bypass, replica_groups=groups,
            ins=[xsb.ap().opt()], outs=[hf[0].ap().opt()])
        for w in range(W):
            rows = mpool.tile([128, D_IN], dt.bfloat16, tag="rows_in")
            nc.sync.dma_start(out=rows[:], in_=xsb[w * 128:(w + 1) * 128, :])
            tp = pp_t.tile([128, 128], dt.bfloat16, tag="tp")
            nc.tensor.transpose(out=tp[:], in_=rows[:], identity=idbf[:])
            nc.vector.tensor_copy(out=hT[0][:, w * 128:(w + 1) * 128], in_=tp[:])

        def do_layer(li, hf_in, hT_in, hT_out, hsl_out, hf_out, last):
            dout = D_OUT if last else D_HID
            stag = stag2[li % 2]
            # ---- phase 1: expand into staging ----
            cur_xc = -1
            xch = None
            cur_pc = -1
            pstrip = None
            chunkb = None
            ps = None
            copy_tog = 0
            for m in range(n_mm):
                b, wsx = mm_block[m], mm_ws[m]
                xc = wsx // CHW
                if xc != cur_xc:
                    cur_xc = xc
                    nw = min(CHW, NWS - xc * CHW)
                    xch = xpool.tile([128, nw, 128], dt.bfloat16, tag="xch",
                                     name="xch")
                    nc.sync.dma_start(
                        out=xch[:, :, :],
                        in_=hf_in[xc * CHW * 128: (xc * CHW + nw) * 128, :]
                        .rearrange("(b p) f -> p b f", p=128))
                pc = m // MMC
                if pc != cur_pc:
                    cur_pc = pc
                    nmm = min(MMC, n_mm - pc * MMC)
                    pvch = ppool.tile([128, nmm * 128], dt.bfloat16,
                                      tag="pvch", name="pvch")
                    nc.sync.dma_start(
                        out=pvch[:],
                        in_=pv_d[:1, pc * MMC * 128: (pc * MMC + nmm) * 128]
                        .partition_broadcast(128))
                    pstrip = ppool.tile([128, nmm * 128], dt.bfloat16,
                                        tag="pstrip", name="pstrip")
                    nc.vector.tensor_tensor(
                        out=pstrip[:],
                        in0=ioct[:, :nmm * 128],
                        in1=pvch[:],
                        op=mybir.AluOpType.is_equal)
                if mm_start[m]:
                    ps = pp_e.tile([128, 128], dt.float32, tag="exp")
                nc.tensor.matmul(
                    out=ps[:],
                    lhsT=pstrip[:, (m - pc * MMC) * 128:(m - pc * MMC + 1) * 128],
                    rhs=xch[:, wsx - xc * CHW, :],
                    start=mm_start[m], stop=mm_stop[m])
                if mm_stop[m]:
                    bb = b % CHB
                    if bb == 0:
                        nb = min(CHB, NBLK - b)
                        chunkb = cpool.tile([128, nb, 128], dt.bfloat16,
                                            tag="chunk", name="chunk")
                    if (b // 8) % 2 == 0:
                        nc.scalar.activation(
                            out=chunkb[:, bb, :], in_=ps[:],
                            func=mybir.ActivationFunctionType.Identity)
                    else:
                        nc.vector.tensor_copy(out=chunkb[:, bb, :], in_=ps[:])
                    if bb == nb - 1 or b == NBLK - 1:
                        b0 = b - bb
                        nc.sync.dma_start(
                            out=stag[b0 * 16:(b0 + bb + 1) * 16, :]
                            .rearrange("(b ph) (pl f) -> (ph pl) b f",
                                       ph=16, pl=8, f=128),
                            in_=chunkb[:, :bb + 1, :])
            # ---- phase 2: collect + aggregate + dense ----
            ixo = 0
            vo = 0
            for g in range(NG):
                nr = NRg[g]
                nt = nrt[g]
                g2 = gpool.tile([128, nt, R * 128], dt.bfloat16, tag="g2",
                                name="g2")
                for k0 in range(0, nr, 512):
                    kn = min(512, nr - k0)
                    nc.gpsimd.dma_gather(
                        g2[:, k0 // 128: (k0 + kn) // 128, :],
                        stag[:, :],
                        idx_sb[:, ixo + k0 // 16: ixo + (k0 + kn) // 16],
                        kn, kn, R * 128)
                ixo += nr // 16
                nw = min(2 * 128, (W - 2 * g) * 128)
                pwin = slice(2 * g * 128, 2 * g * 128 + nw)
                sstrip = spool.tile([128, nt * R, 256], dt.bfloat16,
                                    tag="sstrip", name="sstrip")
                for t in range(nt):
                    nc.vector.tensor_tensor(
                        out=sstrip[:, t * R:(t + 1) * R, :],
                        in0=va_sb[:, vo + t * R: vo + (t + 1) * R]
                        .rearrange("p (j o) -> p j o", o=1)
                        .to_broadcast([128, R, 256]),
                        in1=io2t[:, :].rearrange("p (j c) -> p j c", c=256),
                        op=mybir.AluOpType.is_equal)
                ps2 = pp.tile([128, 256], dt.float32, tag="agg")
                nmm2 = nt * R
                for t in range(nt):
                    for pos in range(R):
                        j = t * R + pos
                        nc.tensor.matmul(
                            out=ps2[:, :nw],
                            lhsT=g2[:, t, pos * 128:(pos + 1) * 128],
                            rhs=sstrip[:, j, :nw],
                            start=(j == 0), stop=(j == nmm2 - 1))
                aggsc = mpool.tile([128, 256], dt.bfloat16, tag="aggsc")
                nc.vector.tensor_tensor(out=aggsc[:, :nw], in0=ps2[:, :nw],
                                        in1=invdeg[:, pwin],
                                        op=mybir.AluOpType.mult)
                pd = pp_d.tile([128, 256], dt.float32, tag="dense")
                nc.tensor.matmul(out=pd[:dout, :nw], lhsT=wl[li][:],
                                 rhs=aggsc[:, :nw], start=True, stop=False)
                nc.tensor.matmul(out=pd[:dout, :nw], lhsT=wr[li][:],
                                 rhs=hT_in[:, pwin], start=False, stop=True)
                if not last:
                    nc.scalar.activation(
                        out=hT_out[:, pwin], in_=pd[:, :nw],
                        func=mybir.ActivationFunctionType.Relu,
                        bias=bias[li][:, :1])
                    for hh in range(nw // 128):
                        w = 2 * g + hh
                        win = slice(w * 128, (w + 1) * 128)
                        tp = pp_t.tile([128, 128], dt.bfloat16, tag="tp")
                        nc.tensor.transpose(out=tp[:], in_=hT_out[:, win],
                                            identity=idbf[:])
                        rows = mpool.tile([128, D_IN], dt.bfloat16,
                                          tag="rows_out")
                        nc.vector.tensor_copy(out=rows[:], in_=tp[:])
                        nc.sync.dma_start(out=hsl_out[win, :], in_=rows[:])
                else:
                    oT = mpool.tile([128, 256], dt.float32, tag="oT")
                    nc.scalar.activation(
                        out=oT[:dout, :nw], in_=pd[:dout, :nw],
                        func=mybir.ActivationFunctionType.Identity,
                        bias=bias[li][:dout, :1])
                    if dout < 128:
                        nc.vector.memset(oT[dout:, :nw], 0.0)
                    for hh in range(nw // 128):
                        w = 2 * g + hh
                        win = slice(w * 128, (w + 1) * 128)
                        tp = pp_t.tile([128, 128], dt.float32, tag="tpf")
                        nc.tensor.transpose(out=tp[:], in_=oT[:, hh * 128:(hh + 1) * 128],
                                            identity=idf[:])
                        negmax = mpool.tile([128, 1], dt.float32, tag="negmax")
                        nc.vector.tensor_reduce(out=negmax[:], in_=tp[:, :D_OUT],
                                                axis=mybir.AxisListType.X,
                                                op=mybir.AluOpType.max, negate=True)
                        esb = mpool.tile([128, D_OUT], dt.float32, tag="esb")
                        nc.scalar.activation(out=esb[:], in_=tp[:, :D_OUT],
                                             func=mybir.ActivationFunctionType.Exp,
                                             bias=negmax[:, :1])
                        ssum = mpool.tile([128, 1], dt.float32, tag="ssum")
                        nc.vector.tensor_reduce(out=ssum[:], in_=esb[:],
                                                axis=mybir.AxisListType.X,
                                                op=mybir.AluOpType.add)
                        lns = mpool.tile([128, 1], dt.float32, tag="lns")
                        nc.scalar.activation(out=lns[:], in_=ssum[:],
                                             func=mybir.ActivationFunctionType.Ln)
                        csub = mpool.tile([128, 1], dt.float32, tag="csub")
                        nc.vector.tensor_tensor(out=csub[:], in0=lns[:], in1=negmax[:],
                                                op=mybir.AluOpType.subtract)
                        res = mpool.tile([128, D_OUT], dt.float32, tag="res")
                        nc.vector.tensor_tensor(out=res[:], in0=tp[:, :D_OUT],
                                                in1=csub[:, :1].to_broadcast([128, D_OUT]),
                                                op=mybir.AluOpType.subtract)
                        nc.sync.dma_start(out=out_d[win, :], in_=res[:])
                vo += nt * R
            if not last:
                nc.gpsimd.collective_compute(
                    "AllGather", mybir.AluOpType.bypass, replica_groups=groups,
                    ins=[hsl_out.ap().opt()], outs=[hf_out.ap().opt()])

        do_layer(0, hf[0], hT[0], hT[1], hsl[0], hf[1], last=False)
        do_layer(1, hf[1], hT[1], hT[0], hsl[1], hf[2], last=False)
        do_layer(2, hf[2], hT[0], None, None, None, last=True)

    nc.compile()
    return nc


def _make_inputs(plan, x, W1_l, W1_r, b1, Wm_l, Wm_r, bm, W2_l, W2_r, b2):
    ident = np.eye(128, dtype=np.float32).astype(BF16)

    def pad_bias(b):
        v = np.zeros((128, 1), np.float32)
        v[: len(b), 0] = np.asarray(b, np.float32)
        return v

    common = {
        "idbf": ident,
        "ioct": np.tile(np.arange(128, dtype=np.float32)[:, None],
                        (1, MMC * 128)).astype(BF16),
        "io2t": np.tile(np.tile(np.arange(256, dtype=np.float32), R)[None, :],
                        (128, 1)).astype(BF16),
        "wl0": np.asarray(W1_l, np.float32).astype(BF16),
        "wr0": np.asarray(W1_r, np.float32).astype(BF16),
        "b0": pad_bias(b1),
        "wl1": np.asarray(Wm_l, np.float32).astype(BF16),
        "wr1": np.asarray(Wm_r, np.float32).astype(BF16),
        "b1": pad_bias(bm),
        "wl2": np.asarray(W2_l, np.float32).astype(BF16),
        "wr2": np.asarray(W2_r, np.float32).astype(BF16),
        "b2": pad_bias(b2),
    }
    x = np.asarray(x, np.float32)
    in_maps = []
    for c in range(N_CORES):
        xs = np.zeros((NPC, D_IN), np.float32)
        xs[:NPC_RAW] = x[c * NPC_RAW:(c + 1) * NPC_RAW]
        m = dict(common)
        m["x_slice"] = xs
        m["invdeg"] = plan["invdeg"][c]
        m["pv"] = plan["PV"][c]
        m["idx"] = np.tile(np.concatenate(
            [plan["idx_w"][g][c] for g in range(NG)], axis=1), (8, 1))
        m["va"] = np.concatenate([plan["VA"][g][c] for g in range(NG)], axis=1)
        in_maps.append(m)
    return in_maps


def _postprocess(results):
    out = np.empty((N, D_OUT), np.float32)
    for c in range(N_CORES):
        out[c * NPC_RAW:(c + 1) * NPC_RAW] = results[c]["out"][:NPC_RAW]
    return out


_CACHE = {}


def kernel(x, edge_index, W1_l, W1_r, b1, Wm_l, Wm_r, bm, W2_l, W2_r, b2,
           _trace=False):
    from concourse.bass_utils import run_bass_kernel_spmd

    edge_index = np.asarray(edge_index)
    key = hash(edge_index.tobytes())
    if key not in _CACHE:
        plan = _plan(edge_index)
        nc = _build(plan)
        _CACHE[key] = (plan, nc)
    plan, nc = _CACHE[key]
    in_maps = _make_inputs(plan, x, W1_l, W1_r, b1, Wm_l, Wm_r, bm,
                           W2_l, W2_r, b2)
    res = run_bass_kernel_spmd(nc, in_maps, core_ids=list(range(N_CORES)),
                               trace=_trace)
    out = _postprocess(res.results)
    if _trace:
        kernel._last_exec_ns = res.exec_time_ns
        kernel._last_res = res
    return out


# revision 9
# speedup vs baseline: 1.0125x; 1.0125x over previous
"""3-layer GraphSAGE (mean agg) on 8 Trainium2 NeuronCores — two-phase radix.

The old per-edge dma_gather spent ~9ns/edge of GPSIMD(Q7) descriptor
generation (3.1ms of a 3.6ms kernel).  Measured on HW: descriptor cost is
per-INDEX, independent of element size.  So:
  Phase 1 (expand): stream the replicated node table sequentially; for each
    128-row table window, a PE one-hot matmul scatters its outgoing edge rows
    into a dst-ordered slot array (8-slot runs, each run pure to one
    dst-window-pair); slots stream to a DRAM staging buffer via large DMAs.
  Phase 2 (collect): per dst-window-pair, dma_gather with elem_size=2KB pulls
    8 edge-rows per descriptor (8x fewer Q7 descriptors); PE aggregates via
    8-position one-hot matmuls into per-window PSUM tiles; then the dense
    SAGE matmuls / ReLU / log_softmax tail as before.
Weights replicated; node features exchanged between layers via AllGather.
"""

import sys
import numpy as np

for _p in ("/opt/trn_rl_repo", "/root/.axon_site/_ro/trn_rl_repo"):
    if _p not in sys.path:
        sys.path.append(_p)

import ml_dtypes

BF16 = ml_dtypes.bfloat16

N = 50000
E = 800000
D_IN = 128
D_HID = 128
D_OUT = 64
N_CORES = 8
WIN = 128
NPC_RAW = N // N_CORES            # 6250
NPC = ((NPC_RAW + WIN - 1) // WIN) * WIN   # 6272
W = NPC // WIN                    # 49
NG = (W + 1) // 2                 # 25 dst-window groups (pairs, last single)
HTAB = N_CORES * NPC              # 50176
NWS = HTAB // WIN                 # 392 table windows
R = 8                             # slots per run (2KB bf16)
CHW = 32                          # table windows per phase-1 chunk
CHB = 16                          # blocks per staging chunk
MMC = 32                          # matmuls per P-strip build


def _plan(edge_index):
    src = np.asarray(edge_index[0], dtype=np.int64)
    dst = np.asarray(edge_index[1], dtype=np.int64)
    ec = dst // NPC_RAW
    rank = dst % NPC_RAW
    ew = rank // WIN
    dl = rank % WIN
    grp = ew // 2
    h = ew % 2
    ep = (src // NPC_RAW) * NPC + (src % NPC_RAW)
    ws = ep // WIN
    sl = ep % WIN

    cnt = np.zeros((N_CORES, NWS, NG), np.int64)
    np.add.at(cnt, (ec, ws, grp), 1)
    runs_cell = -(-cnt // R)                      # [8, NWS, NG]
    # slots per (core, ws): run-aligned
    s_c_ws = runs_cell.sum(axis=2) * R            # [8, NWS]
    REG = s_c_ws.max(axis=0)                      # [NWS] (multiple of 8)
    RB = np.concatenate([[0], np.cumsum(REG)])    # region base slots
    LSLOT = int(RB[-1])
    NBLK = -(-LSLOT // 128)
    LSLOT_PAD = NBLK * 128
    NRUN = LSLOT_PAD // R
    assert NRUN < 32768, f"run index overflow: {NRUN}"

    # per-core cell base slot
    cum_runs = np.cumsum(runs_cell, axis=2) - runs_cell    # exclusive, [8,NWS,NG]
    cb = RB[None, :NWS, None] + cum_runs * R               # [8, NWS, NG]

    # edge slot assignment
    okey = np.lexsort((dl, grp, ws, ec))
    ec_s, ws_s, grp_s, sl_s, dl_s, h_s = (a[okey] for a in (ec, ws, grp, sl, dl, h))
    cell_id = (ec_s * NWS + ws_s) * NG + grp_s
    newcell = np.concatenate([[True], cell_id[1:] != cell_id[:-1]])
    gstart = np.maximum.accumulate(np.where(newcell, np.arange(E), 0))
    krank = np.arange(E) - gstart
    slot = cb[ec_s, ws_s, grp_s] + krank

    srcl = np.full((N_CORES, LSLOT_PAD), -1.0, np.float32)
    dA = np.full((N_CORES, LSLOT_PAD), -1.0, np.float32)
    srcl[ec_s, slot] = sl_s
    dA[ec_s, slot] = dl_s + 128.0 * h_s

    # matmul descriptor list: blocks split into region segments
    reg_end = RB[1:]
    mm_block, mm_ws, mm_start, mm_stop, mm_lo, mm_hi = [], [], [], [], [], []
    for b in range(NBLK):
        lo, hi = b * 128, (b + 1) * 128
        wlo = int(np.searchsorted(reg_end, lo, side="right"))
        segs = []
        cur = lo
        wcur = min(wlo, NWS - 1)
        while cur < hi:
            seg_end = min(hi, int(reg_end[wcur]) if wcur < NWS else hi)
            if seg_end <= cur:       # dead tail past LSLOT
                seg_end = hi
            segs.append((wcur, cur, seg_end))
            cur = seg_end
            wcur = min(wcur + 1, NWS - 1)
        for i, (wsx, lo2, hi2) in enumerate(segs):
            mm_block.append(b)
            mm_ws.append(wsx)
            mm_start.append(i == 0)
            mm_stop.append(i == len(segs) - 1)
            mm_lo.append(lo2 - lo)
            mm_hi.append(hi2 - lo)
    n_mm = len(mm_block)

    PV = np.full((N_CORES, n_mm * 128), -1.0, np.float32)
    for m in range(n_mm):
        b, lo2, hi2 = mm_block[m], mm_lo[m], mm_hi[m]
        PV[:, m * 128 + lo2: m * 128 + hi2] = \
            srcl[:, b * 128 + lo2: b * 128 + hi2]

    # phase-2 run lists per group
    NRg, nrt, idx_w, VA = [], [], [], []
    for g in range(NG):
        rls = []
        for c in range(N_CORES):
            parts = []
            for wsx in range(NWS):
                k = int(runs_cell[c, wsx, g])
                if k:
                    base = int(cb[c, wsx, g]) // R
                    parts.append(base + np.arange(k))
            rl = np.concatenate(parts) if parts else np.zeros(0, np.int64)
            rls.append(rl)
        mx = max(len(r) for r in rls)
        nr = ((mx + 127) // 128) * 128
        NRg.append(nr)
        nrt.append(nr // 128)
        iw, va = [], []
        for c in range(N_CORES):
            rl = np.zeros(nr, np.int64)
            rl[: len(rls[c])] = rls[c]
            # wrap for dma_gather: per 512-call columns of reshape(-1,16).T
            cols = []
            for b0 in range(0, nr, 512):
                blk = rl[b0: b0 + 512]
                cols.append(blk.reshape(-1, 16).T)
            iw.append(np.concatenate(cols, axis=1).astype(np.int16))
            a = dA[c][(rl[:, None] * R + np.arange(R))].astype(np.float32)
            if len(rls[c]) < nr:    # padded runs contribute nothing
                a[len(rls[c]):] = -1.0
            va.append(a.reshape(nr // 128, 128, R).transpose(1, 0, 2)
                      .reshape(128, -1).astype(BF16))
        idx_w.append(iw)
        VA.append(va)

    deg = np.bincount(dst, minlength=N).astype(np.float32)
    invdeg = []
    for c in range(N_CORES):
        v = np.ones(NPC, np.float32)
        v[:NPC_RAW] = 1.0 / np.maximum(deg[c * NPC_RAW:(c + 1) * NPC_RAW], 1.0)
        invdeg.append(np.tile(v[None, :], (WIN, 1)))

    return {
        "LSLOT_PAD": LSLOT_PAD, "NBLK": NBLK, "NRUN": NRUN, "n_mm": n_mm,
        "mm_block": mm_block, "mm_ws": mm_ws, "mm_start": mm_start,
        "mm_stop": mm_stop, "NRg": NRg, "nrt": nrt,
        "PV": [np.ascontiguousarray(PV[c]).astype(BF16)[None, :] for c in range(N_CORES)],
        "idx_w": idx_w, "VA": VA, "invdeg": invdeg,
    }


def _build(plan):
    import concourse.bacc as bacc
    import concourse.bass as bass
    import concourse.mybir as mybir
    import concourse.tile as tile
    from contextlib import ExitStack

    dt = mybir.dt
    NBLK, NRUN, n_mm = plan["NBLK"], plan["NRUN"], plan["n_mm"]
    NRg, nrt = plan["NRg"], plan["nrt"]
    mm_block, mm_ws = plan["mm_block"], plan["mm_ws"]
    mm_start, mm_stop = plan["mm_start"], plan["mm_stop"]
    SNRT = sum(nrt)

    nc = bacc.Bacc("TRN2", target_bir_lowering=False)

    x_sl = nc.dram_tensor("x_slice", [NPC, D_IN], dt.float32, kind="ExternalInput")
    pv_d = nc.dram_tensor("pv", [1, n_mm * 128], dt.bfloat16, kind="ExternalInput")
    idx_d = nc.dram_tensor("idx", [128, (sum(NRg)) // 16], dt.int16,
                           kind="ExternalInput")
    va_d = nc.dram_tensor("va", [128, SNRT * R], dt.bfloat16, kind="ExternalInput")
    invdeg_d = nc.dram_tensor("invdeg", [128, NPC], dt.float32, kind="ExternalInput")
    idbf_d = nc.dram_tensor("idbf", [128, 128], dt.bfloat16, kind="ExternalInput")
    ioct_d = nc.dram_tensor("ioct", [128, MMC * 128], dt.bfloat16,
                            kind="ExternalInput")
    io2t_d = nc.dram_tensor("io2t", [128, R * 256], dt.bfloat16,
                            kind="ExternalInput")
    wl_d, wr_d, b_d = [], [], []
    for li, (din, dout) in enumerate([(D_IN, D_HID), (D_HID, D_HID),
                                      (D_HID, D_OUT)]):
        wl_d.append(nc.dram_tensor(f"wl{li}", [din, dout], dt.bfloat16,
                                   kind="ExternalInput"))
        wr_d.append(nc.dram_tensor(f"wr{li}", [din, dout], dt.bfloat16,
                                   kind="ExternalInput"))
        b_d.append(nc.dram_tensor(f"b{li}", [128, 1], dt.float32,
                                  kind="ExternalInput"))

    xsb = nc.dram_tensor("xsb", [NPC, D_IN], dt.bfloat16)
    hsl = [nc.dram_tensor(f"hsl{i}", [NPC, D_IN], dt.bfloat16) for i in range(2)]
    hf = [nc.dram_tensor(f"hf{i}", [HTAB, D_IN], dt.bfloat16) for i in range(3)]
    stag2 = [nc.dram_tensor(f"stag{i}", [NRUN, R * 128], dt.bfloat16)
             for i in range(2)]
    out_d = nc.dram_tensor("out", [NPC, D_OUT], dt.float32, kind="ExternalOutput")

    groups = [list(range(N_CORES))]

    with tile.TileContext(nc) as tc, ExitStack() as ctx:
        per = ctx.enter_context(tc.tile_pool(name="persist", bufs=1))
        xpool = ctx.enter_context(tc.tile_pool(name="xch", bufs=2))
        ppool = ctx.enter_context(tc.tile_pool(name="pstrip", bufs=2))
        cpool = ctx.enter_context(tc.tile_pool(name="chunk", bufs=3))
        gpool = ctx.enter_context(tc.tile_pool(name="g2", bufs=2))
        spool = ctx.enter_context(tc.tile_pool(name="sstrip", bufs=2))
        mpool = ctx.enter_context(tc.tile_pool(name="misc", bufs=2))
        pp_e = ctx.enter_context(tc.tile_pool(name="ps_exp", bufs=2, space="PSUM"))
        pp = ctx.enter_context(tc.tile_pool(name="ps_agg", bufs=2, space="PSUM"))
        pp_d = ctx.enter_context(tc.tile_pool(name="ps_dense", bufs=2, space="PSUM"))
        pp_t = ctx.enter_context(tc.tile_pool(name="ps_tr", bufs=1, space="PSUM"))

        def load_const(dram, shape, dtp, tag):
            t = per.tile(shape, dtp, tag=tag, name=tag)
            nc.sync.dma_start(out=t[:], in_=dram[:])
            return t

        idbf = load_const(idbf_d, [128, 128], dt.bfloat16, "idbf")
        ioct = load_const(ioct_d, [128, MMC * 128], dt.bfloat16, "ioct")
        io2t = load_const(io2t_d, [128, R * 256], dt.bfloat16, "io2t")
        invdeg = load_const(invdeg_d, [128, NPC], dt.float32, "invdeg")
        wl = [load_const(wl_d[i], list(wl_d[i].shape), dt.bfloat16, f"wl{i}")
              for i in range(3)]
        wr = [load_const(wr_d[i], list(wr_d[i].shape), dt.bfloat16, f"wr{i}")
              for i in range(3)]
        bias = [load_const(b_d[i], [128, 1], dt.float32, f"b{i}") for i in range(3)]
        idx_sb = load_const(idx_d, [128, sum(NRg) // 16], dt.int16, "idx")
        va_sb = load_const(va_d, [128, SNRT * R], dt.bfloat16, "va")

        # iotas
        iota_col = per.tile([128, 1], dt.bfloat16, tag="iotac")
        nc.gpsimd.iota(iota_col[:], pattern=[[1, 1]], base=0,
                       channel_multiplier=1, allow_small_or_imprecise_dtypes=True)
        iota_row = per.tile([128, 128], dt.bfloat16, tag="iotar")
        nc.gpsimd.iota(iota_row[:], pattern=[[1, 128]], base=0,
                       channel_multiplier=0, allow_small_or_imprecise_dtypes=True)
        iota2t_d = None
        zbias = per.tile([128, 1], dt.float32, tag="zbias")
        nc.gpsimd.memset(zbias[:], 0.0)

        idf = per.tile([128, 128], dt.float32, tag="idf")
        nc.vector.tensor_copy(out=idf[:], in_=idbf[:])
        hT = [per.tile([128, NPC], dt.bfloat16, tag=f"hT{i}", name=f"hT{i}")
              for i in range(2)]

        # bootstrap: bf16 cast + AllGather + hT[0]
        nc.gpsimd.dma_start(out=xsb[:, :], in_=x_sl[:, :])
        nc.gpsimd.collective_compute(
            "AllGather", mybir.AluOpType.bypass, replica_groups=groups,
            ins=[xsb.ap().opt()], outs=[hf[0].ap().opt()])
        for w in range(W):
            rows = mpool.tile([128, D_IN], dt.bfloat16, tag="rows_in")
            nc.sync.dma_start(out=rows[:], in_=xsb[w * 128:(w + 1) * 128, :])
            tp = pp_t.tile([128, 128], dt.bfloat16, tag="tp")
            nc.tensor.transpose(out=tp[:], in_=rows[:], identity=idbf[:])
            nc.vector.tensor_copy(out=hT[0][:, w * 128:(w + 1) * 128], in_=tp[:])

        def do_layer(li, hf_in, hT_in, hT_out, hsl_out, hf_out, last):
            dout = D_OUT if last else D_HID
            stag = stag2[li % 2]
            # ---- phase 1: expand into staging ----
            cur_xc = -1
            xch = None
            cur_pc = -1
            pstrip = None
            chunkb = None
            ps = None
            copy_tog = 0
            for m in range(n_mm):
                b, wsx = mm_block[m], mm_ws[m]
                xc = wsx // CHW
                if xc != cur_xc:
                    cur_xc = xc
                    nw = min(CHW, NWS - xc * CHW)
                    xch = xpool.tile([128, nw, 128], dt.bfloat16, tag="xch",
                                     name="xch")
                    nc.sync.dma_start(
                        out=xch[:, :, :],
                        in_=hf_in[xc * CHW * 128: (xc * CHW + nw) * 128, :]
                        .rearrange("(b p) f -> p b f", p=128))
                pc = m // MMC
                if pc != cur_pc:
                    cur_pc = pc
                    nmm = min(MMC, n_mm - pc * MMC)
                    pvch = ppool.tile([128, nmm * 128], dt.bfloat16,
                                      tag="pvch", name="pvch")
                    nc.sync.dma_start(
                        out=pvch[:],
                        in_=pv_d[:1, pc * MMC * 128: (pc * MMC + nmm) * 128]
                        .partition_broadcast(128))
                    pstrip = ppool.tile([128, nmm * 128], dt.bfloat16,
                                        tag="pstrip", name="pstrip")
                    nc.vector.tensor_tensor(
                        out=pstrip[:],
                        in0=ioct[:, :nmm * 128],
                        in1=pvch[:],
                        op=mybir.AluOpType.is_equal)
                if mm_start[m]:
                    ps = pp_e.tile([128, 128], dt.float32, tag="exp")
                nc.tensor.matmul(
                    out=ps[:],
                    lhsT=pstrip[:, (m - pc * MMC) * 128:(m - pc * MMC + 1) * 128],
                    rhs=xch[:, wsx - xc * CHW, :],
                    start=mm_start[m], stop=mm_stop[m])
                if mm_stop[m]:
                    bb = b % CHB
                    if bb == 0:
                        nb = min(CHB, NBLK - b)
                        chunkb = cpool.tile([128, nb, 128], dt.bfloat16,
                                            tag="chunk", name="chunk")
                    nc.scalar.activation(
                        out=chunkb[:, bb, :], in_=ps[:],
                        func=mybir.ActivationFunctionType.Identity,
                        bias=zbias[:, :1])
                    if bb == nb - 1 or b == NBLK - 1:
                        b0 = b - bb
                        nc.sync.dma_start(
                            out=stag[b0 * 16:(b0 + bb + 1) * 16, :]
                            .rearrange("(b ph) (pl f) -> (ph pl) b f",
                                       ph=16, pl=8, f=128),
                            in_=chunkb[:, :bb + 1, :])
            # ---- phase 2: collect + aggregate + dense ----
            ixo = 0
            vo = 0
            for g in range(NG):
                nr = NRg[g]
                nt = nrt[g]
                g2 = gpool.tile([128, nt, R * 128], dt.bfloat16, tag="g2",
                                name="g2")
                for k0 in range(0, nr, 512):
                    kn = min(512, nr - k0)
                    nc.gpsimd.dma_gather(
                        g2[:, k0 // 128: (k0 + kn) // 128, :],
                        stag[:, :],
                        idx_sb[:, ixo + k0 // 16: ixo + (k0 + kn) // 16],
                        kn, kn, R * 128)
                ixo += nr // 16
                nw = min(2 * 128, (W - 2 * g) * 128)
                pwin = slice(2 * g * 128, 2 * g * 128 + nw)
                sstrip = spool.tile([128, nt * R, 256], dt.bfloat16,
                                    tag="sstrip", name="sstrip")
                for t in range(nt):
                    nc.vector.tensor_tensor(
                        out=sstrip[:, t * R:(t + 1) * R, :],
                        in0=va_sb[:, vo + t * R: vo + (t + 1) * R]
                        .rearrange("p (j o) -> p j o", o=1)
                        .to_broadcast([128, R, 256]),
                        in1=io2t[:, :].rearrange("p (j c) -> p j c", c=256),
                        op=mybir.AluOpType.is_equal)
                ps2 = pp.tile([128, 256], dt.float32, tag="agg")
                nmm2 = nt * R
                for t in range(nt):
                    for pos in range(R):
                        j = t * R + pos
                        nc.tensor.matmul(
                            out=ps2[:, :nw],
                            lhsT=g2[:, t, pos * 128:(pos + 1) * 128],
                            rhs=sstrip[:, j, :nw],
                            start=(j == 0), stop=(j == nmm2 - 1))
                aggsc = mpool.tile([128, 256], dt.bfloat16, tag="aggsc")
                nc.vector.tensor_tensor(out=aggsc[:, :nw], in0=ps2[:, :nw],
                                        in1=invdeg[:, pwin],
                                        op=mybir.AluOpType.mult)
                pd = pp_d.tile([128, 256], dt.float32, tag="dense")
                nc.tensor.matmul(out=pd[:dout, :nw], lhsT=wl[li][:],
                                 rhs=aggsc[:, :nw], start=True, stop=False)
                nc.tensor.matmul(out=pd[:dout, :nw], lhsT=wr[li][:],
                                 rhs=hT_in[:, pwin], start=False, stop=True)
                if not last:
                    nc.scalar.activation(
                        out=hT_out[:, pwin], in_=pd[:, :nw],
                        func=mybir.ActivationFunctionType.Relu,
                        bias=bias[li][:, :1])
                    for hh in range(nw // 128):
                        w = 2 * g + hh
                        win = slice(w * 128, (w + 1) * 128)
                        tp = pp_t.tile([128, 128], dt.bfloat16, tag="tp")
                        nc.tensor.transpose(out=tp[:], in_=hT_out[:, win],
                                            identity=idbf[:])
                        rows = mpool.tile([128, D_IN], dt.bfloat16,
                                          tag="rows_out")
                        nc.vector.tensor_copy(out=rows[:], in_=tp[:])
                        nc.sync.dma_start(out=hsl_out[win, :], in_=rows[:])
                else:
                    oT = mpool.tile([128, 256], dt.float32, tag="oT")
                    nc.scalar.activation(
                        out=oT[:dout, :nw], in_=pd[:dout, :nw],
                        func=mybir.ActivationFunctionType.Identity,
                        bias=bias[li][:dout, :1])
                    if dout < 128:
                        nc.vector.memset(oT[dout:, :nw], 0.0)
                    for hh in range(nw // 128):
                        w = 2 * g + hh
                        win = slice(w * 128, (w + 1) * 128)
                        tp = pp_t.tile([128, 128], dt.float32, tag="tpf")
                        nc.tensor.transpose(out=tp[:], in_=oT[:, hh * 128:(hh + 1) * 128],
                                            identity=idf[:])
                        negmax = mpool.tile([128, 1], dt.float32, tag="negmax")
                        nc.vector.tensor_reduce(out=negmax[:], in_=tp[:, :D_OUT],
                                                axis=mybir.AxisListType.X,
                                                op=mybir.AluOpType.max, negate=True)
                        esb = mpool.tile([128, D_OUT], dt.float32, tag="esb")
                        nc.scalar.activation(out=esb[:], in_=tp[:, :D_OUT],
                                             func=mybir.ActivationFunctionType.Exp,
                                             bias=negmax[:, :1])
                        ssum = mpool.tile([128, 1], dt.float32, tag="ssum")
                        nc.vector.tensor_reduce(out=ssum[:], in_=esb[:],
                                                axis=mybir.AxisListType.X,
                                                op=mybir.AluOpType.add)
                        lns = mpool.tile([128, 1], dt.float32, tag="lns")
                        nc.scalar.activation(out=lns[:], in_=ssum[:],
                                             func=mybir.ActivationFunctionType.Ln)
                        csub = mpool.tile([128, 1], dt.float32, tag="csub")
                        nc.vector.tensor_tensor(out=csub[:], in0=lns[:], in1=negmax[:],
                                                op=mybir.AluOpType.subtract)
                        res = mpool.tile([128, D_OUT], dt.float32, tag="res")
                        nc.vector.tensor_tensor(out=res[:], in0=tp[:, :D_OUT],
                                                in1=csub[:, :1].to_broadcast([128, D_OUT]),
                                                op=mybir.AluOpType.subtract)
                        nc.sync.dma_start(out=out_d[win, :], in_=res[:])
                vo += nt * R
            if not last:
                nc.gpsimd.collective_compute(
                    "AllGather", mybir.AluOpType.bypass, replica_groups=groups,
                    ins=[hsl_out.ap().opt()], outs=[hf_out.ap().opt()])

        do_layer(0, hf[0], hT[0], hT[1], hsl[0], hf[1], last=False)
        do_layer(1, hf[1], hT[1], hT[0], hsl[1], hf[2], last=False)
        do_layer(2, hf[2], hT[0], None, None, None, last=True)

    nc.compile()
    return nc


def _make_inputs(plan, x, W1_l, W1_r, b1, Wm_l, Wm_r, bm, W2_l, W2_r, b2):
    ident = np.eye(128, dtype=np.float32).astype(BF16)

    def pad_bias(b):
        v = np.zeros((128, 1), np.float32)
        v[: len(b), 0] = np.asarray(b, np.float32)
        return v

    common = {
        "idbf": ident,
        "ioct": np.tile(np.arange(128, dtype=np.float32)[:, None],
                        (1, MMC * 128)).astype(BF16),
        "io2t": np.tile(np.tile(np.arange(256, dtype=np.float32), R)[None, :],
                        (128, 1)).astype(BF16),
        "wl0": np.asarray(W1_l, np.float32).astype(BF16),
        "wr0": np.asarray(W1_r, np.float32).astype(BF16),
        "b0": pad_bias(b1),
        "wl1": np.asarray(Wm_l, np.float32).astype(BF16),
        "wr1": np.asarray(Wm_r, np.float32).astype(BF16),
        "b1": pad_bias(bm),
        "wl2": np.asarray(W2_l, np.float32).astype(BF16),
        "wr2": np.asarray(W2_r, np.float32).astype(BF16),
        "b2": pad_bias(b2),
    }
    x = np.asarray(x, np.float32)
    in_maps = []
    for c in range(N_CORES):
        xs = np.zeros((NPC, D_IN), np.float32)
        xs[:NPC_RAW] = x[c * NPC_RAW:(c + 1) * NPC_RAW]
        m = dict(common)
        m["x_slice"] = xs
        m["invdeg"] = plan["invdeg"][c]
        m["pv"] = plan["PV"][c]
        m["idx"] = np.tile(np.concatenate(
            [plan["idx_w"][g][c] for g in range(NG)], axis=1), (8, 1))
        m["va"] = np.concatenate([plan["VA"][g][c] for g in range(NG)], axis=1)
        in_maps.append(m)
    return in_maps


def _postprocess(results):
    out = np.empty((N, D_OUT), np.float32)
    for c in range(N_CORES):
        out[c * NPC_RAW:(c + 1) * NPC_RAW] = results[c]["out"][:NPC_RAW]
    return out


_CACHE = {}


def kernel(x, edge_index, W1_l, W1_r, b1, Wm_l, Wm_r, bm, W2_l, W2_r, b2,
           _trace=False):
    from concourse.bass_utils import run_bass_kernel_spmd

    edge_index = np.asarray(edge_index)
    key = hash(edge_index.tobytes())
    if key not in _CACHE:
        plan = _plan(edge_index)
        nc = _build(plan)
        _CACHE[key] = (plan, nc)
    plan, nc = _CACHE[key]
    in_maps = _make_inputs(plan, x, W1_l, W1_r, b1, Wm_l, Wm_r, bm,
                           W2_l, W2_r, b2)
    res = run_bass_kernel_spmd(nc, in_maps, core_ids=list(range(N_CORES)),
                               trace=_trace)
    out = _postprocess(res.results)
    if _trace:
        kernel._last_exec_ns = res.exec_time_ns
        kernel._last_res = res
    return out


# revision 10
# speedup vs baseline: 1.0251x; 1.0124x over previous
"""3-layer GraphSAGE (mean agg) on 8 Trainium2 NeuronCores — two-phase radix.

The old per-edge dma_gather spent ~9ns/edge of GPSIMD(Q7) descriptor
generation (3.1ms of a 3.6ms kernel).  Measured on HW: descriptor cost is
per-INDEX, independent of element size.  So:
  Phase 1 (expand): stream the replicated node table sequentially; for each
    128-row table window, a PE one-hot matmul scatters its outgoing edge rows
    into a dst-ordered slot array (8-slot runs, each run pure to one
    dst-window-pair); slots stream to a DRAM staging buffer via large DMAs.
  Phase 2 (collect): per dst-window-pair, dma_gather with elem_size=2KB pulls
    8 edge-rows per descriptor (8x fewer Q7 descriptors); PE aggregates via
    8-position one-hot matmuls into per-window PSUM tiles; then the dense
    SAGE matmuls / ReLU / log_softmax tail as before.
Weights replicated; node features exchanged between layers via AllGather.
"""

import sys
import numpy as np

for _p in ("/opt/trn_rl_repo", "/root/.axon_site/_ro/trn_rl_repo"):
    if _p not in sys.path:
        sys.path.append(_p)

import ml_dtypes

BF16 = ml_dtypes.bfloat16

N = 50000
E = 800000
D_IN = 128
D_HID = 128
D_OUT = 64
N_CORES = 8
WIN = 128
NPC_RAW = N // N_CORES            # 6250
NPC = ((NPC_RAW + WIN - 1) // WIN) * WIN   # 6272
W = NPC // WIN                    # 49
NG = (W + 1) // 2                 # 25 dst-window groups (pairs, last single)
HTAB = N_CORES * NPC              # 50176
NWS = HTAB // WIN                 # 392 table windows
R = 8                             # slots per run (2KB bf16)
CHW = 32                          # table windows per phase-1 chunk
CHB = 16                          # blocks per staging chunk
MMC = 32                          # matmuls per P-strip build


def _plan(edge_index):
    src = np.asarray(edge_index[0], dtype=np.int64)
    dst = np.asarray(edge_index[1], dtype=np.int64)
    ec = dst // NPC_RAW
    rank = dst % NPC_RAW
    ew = rank // WIN
    dl = rank % WIN
    grp = ew // 2
    h = ew % 2
    ep = (src // NPC_RAW) * NPC + (src % NPC_RAW)
    ws = ep // WIN
    sl = ep % WIN

    cnt = np.zeros((N_CORES, NWS, NG), np.int64)
    np.add.at(cnt, (ec, ws, grp), 1)
    runs_cell = -(-cnt // R)                      # [8, NWS, NG]
    # slots per (core, ws): run-aligned
    s_c_ws = runs_cell.sum(axis=2) * R            # [8, NWS]
    REG = s_c_ws.max(axis=0)                      # [NWS] (multiple of 8)
    RB = np.concatenate([[0], np.cumsum(REG)])    # region base slots
    LSLOT = int(RB[-1])
    NBLK = -(-LSLOT // 128)
    LSLOT_PAD = NBLK * 128
    NRUN = LSLOT_PAD // R
    assert NRUN < 32768, f"run index overflow: {NRUN}"

    # per-core cell base slot
    cum_runs = np.cumsum(runs_cell, axis=2) - runs_cell    # exclusive, [8,NWS,NG]
    cb = RB[None, :NWS, None] + cum_runs * R               # [8, NWS, NG]

    # edge slot assignment
    okey = np.lexsort((dl, grp, ws, ec))
    ec_s, ws_s, grp_s, sl_s, dl_s, h_s = (a[okey] for a in (ec, ws, grp, sl, dl, h))
    cell_id = (ec_s * NWS + ws_s) * NG + grp_s
    newcell = np.concatenate([[True], cell_id[1:] != cell_id[:-1]])
    gstart = np.maximum.accumulate(np.where(newcell, np.arange(E), 0))
    krank = np.arange(E) - gstart
    slot = cb[ec_s, ws_s, grp_s] + krank

    srcl = np.full((N_CORES, LSLOT_PAD), -1.0, np.float32)
    dA = np.full((N_CORES, LSLOT_PAD), -1.0, np.float32)
    srcl[ec_s, slot] = sl_s
    dA[ec_s, slot] = dl_s + 128.0 * h_s

    # matmul descriptor list: blocks split into region segments
    reg_end = RB[1:]
    mm_block, mm_ws, mm_start, mm_stop, mm_lo, mm_hi = [], [], [], [], [], []
    for b in range(NBLK):
        lo, hi = b * 128, (b + 1) * 128
        wlo = int(np.searchsorted(reg_end, lo, side="right"))
        segs = []
        cur = lo
        wcur = min(wlo, NWS - 1)
        while cur < hi:
            seg_end = min(hi, int(reg_end[wcur]) if wcur < NWS else hi)
            if seg_end <= cur:       # dead tail past LSLOT
                seg_end = hi
            segs.append((wcur, cur, seg_end))
            cur = seg_end
            wcur = min(wcur + 1, NWS - 1)
        for i, (wsx, lo2, hi2) in enumerate(segs):
            mm_block.append(b)
            mm_ws.append(wsx)
            mm_start.append(i == 0)
            mm_stop.append(i == len(segs) - 1)
            mm_lo.append(lo2 - lo)
            mm_hi.append(hi2 - lo)
    n_mm = len(mm_block)

    PV = np.full((N_CORES, n_mm * 128), -1.0, np.float32)
    for m in range(n_mm):
        b, lo2, hi2 = mm_block[m], mm_lo[m], mm_hi[m]
        PV[:, m * 128 + lo2: m * 128 + hi2] = \
            srcl[:, b * 128 + lo2: b * 128 + hi2]

    # phase-2 run lists per group
    NRg, nrt, idx_w, VA = [], [], [], []
    for g in range(NG):
        rls = []
        for c in range(N_CORES):
            parts = []
            for wsx in range(NWS):
                k = int(runs_cell[c, wsx, g])
                if k:
                    base = int(cb[c, wsx, g]) // R
                    parts.append(base + np.arange(k))
            rl = np.concatenate(parts) if parts else np.zeros(0, np.int64)
            rls.append(rl)
        mx = max(len(r) for r in rls)
        nr = ((mx + 127) // 128) * 128
        NRg.append(nr)
        nrt.append(nr // 128)
        iw, va = [], []
        for c in range(N_CORES):
            rl = np.zeros(nr, np.int64)
            rl[: len(rls[c])] = rls[c]
            # wrap for dma_gather: per 512-call columns of reshape(-1,16).T
            cols = []
            for b0 in range(0, nr, 512):
                blk = rl[b0: b0 + 512]
                cols.append(blk.reshape(-1, 16).T)
            iw.append(np.concatenate(cols, axis=1).astype(np.int16))
            a = dA[c][(rl[:, None] * R + np.arange(R))].astype(np.float32)
            if len(rls[c]) < nr:    # padded runs contribute nothing
                a[len(rls[c]):] = -1.0
            va.append(a.reshape(nr // 128, 128, R).transpose(1, 0, 2)
                      .reshape(128, -1).astype(BF16))
        idx_w.append(iw)
        VA.append(va)

    deg = np.bincount(dst, minlength=N).astype(np.float32)
    invdeg = []
    for c in range(N_CORES):
        v = np.ones(NPC, np.float32)
        v[:NPC_RAW] = 1.0 / np.maximum(deg[c * NPC_RAW:(c + 1) * NPC_RAW], 1.0)
        invdeg.append(np.tile(v[None, :], (WIN, 1)))

    return {
        "LSLOT_PAD": LSLOT_PAD, "NBLK": NBLK, "NRUN": NRUN, "n_mm": n_mm,
        "mm_block": mm_block, "mm_ws": mm_ws, "mm_start": mm_start,
        "mm_stop": mm_stop, "NRg": NRg, "nrt": nrt,
        "PV": [np.ascontiguousarray(PV[c]).astype(BF16)[None, :] for c in range(N_CORES)],
        "idx_w": idx_w, "VA": VA, "invdeg": invdeg,
    }


def _build(plan):
    import concourse.bacc as bacc
    import concourse.bass as bass
    import concourse.mybir as mybir
    import concourse.tile as tile
    from contextlib import ExitStack

    dt = mybir.dt
    NBLK, NRUN, n_mm = plan["NBLK"], plan["NRUN"], plan["n_mm"]
    NRg, nrt = plan["NRg"], plan["nrt"]
    mm_block, mm_ws = plan["mm_block"], plan["mm_ws"]
    mm_start, mm_stop = plan["mm_start"], plan["mm_stop"]
    SNRT = sum(nrt)

    nc = bacc.Bacc("TRN2", target_bir_lowering=False)

    x_sl = nc.dram_tensor("x_slice", [NPC, D_IN], dt.float32, kind="ExternalInput")
    pv_d = nc.dram_tensor("pv", [1, n_mm * 128], dt.bfloat16, kind="ExternalInput")
    idx_d = nc.dram_tensor("idx", [128, (sum(NRg)) // 16], dt.int16,
                           kind="ExternalInput")
    va_d = nc.dram_tensor("va", [128, SNRT * R], dt.bfloat16, kind="ExternalInput")
    invdeg_d = nc.dram_tensor("invdeg", [128, NPC], dt.float32, kind="ExternalInput")
    idbf_d = nc.dram_tensor("idbf", [128, 128], dt.bfloat16, kind="ExternalInput")
    ioct_d = nc.dram_tensor("ioct", [128, MMC * 128], dt.bfloat16,
                            kind="ExternalInput")
    io2t_d = nc.dram_tensor("io2t", [128, R * 256], dt.bfloat16,
                            kind="ExternalInput")
    wl_d, wr_d, b_d = [], [], []
    for li, (din, dout) in enumerate([(D_IN, D_HID), (D_HID, D_HID),
                                      (D_HID, D_OUT)]):
        wl_d.append(nc.dram_tensor(f"wl{li}", [din, dout], dt.bfloat16,
                                   kind="ExternalInput"))
        wr_d.append(nc.dram_tensor(f"wr{li}", [din, dout], dt.bfloat16,
                                   kind="ExternalInput"))
        b_d.append(nc.dram_tensor(f"b{li}", [128, 1], dt.float32,
                                  kind="ExternalInput"))

    xsb = nc.dram_tensor("xsb", [NPC, D_IN], dt.bfloat16)
    hsl = [nc.dram_tensor(f"hsl{i}", [NPC, D_IN], dt.bfloat16) for i in range(2)]
    hf = [nc.dram_tensor(f"hf{i}", [HTAB, D_IN], dt.bfloat16) for i in range(3)]
    stag2 = [nc.dram_tensor(f"stag{i}", [NRUN, R * 128], dt.bfloat16)
             for i in range(2)]
    out_d = nc.dram_tensor("out", [NPC, D_OUT], dt.float32, kind="ExternalOutput")

    groups = [list(range(N_CORES))]

    with tile.TileContext(nc) as tc, ExitStack() as ctx:
        per = ctx.enter_context(tc.tile_pool(name="persist", bufs=1))
        xpool = ctx.enter_context(tc.tile_pool(name="xch", bufs=2))
        ppool = ctx.enter_context(tc.tile_pool(name="pstrip", bufs=2))
        cpool = ctx.enter_context(tc.tile_pool(name="chunk", bufs=3))
        gpool = ctx.enter_context(tc.tile_pool(name="g2", bufs=2))
        spool = ctx.enter_context(tc.tile_pool(name="sstrip", bufs=2))
        mpool = ctx.enter_context(tc.tile_pool(name="misc", bufs=2))
        pp_e = ctx.enter_context(tc.tile_pool(name="ps_exp", bufs=2, space="PSUM"))
        pp = ctx.enter_context(tc.tile_pool(name="ps_agg", bufs=2, space="PSUM"))
        pp_d = ctx.enter_context(tc.tile_pool(name="ps_dense", bufs=2, space="PSUM"))
        pp_t = ctx.enter_context(tc.tile_pool(name="ps_tr", bufs=1, space="PSUM"))

        def load_const(dram, shape, dtp, tag):
            t = per.tile(shape, dtp, tag=tag, name=tag)
            nc.sync.dma_start(out=t[:], in_=dram[:])
            return t

        idbf = load_const(idbf_d, [128, 128], dt.bfloat16, "idbf")
        ioct = load_const(ioct_d, [128, MMC * 128], dt.bfloat16, "ioct")
        io2t = load_const(io2t_d, [128, R * 256], dt.bfloat16, "io2t")
        invdeg = load_const(invdeg_d, [128, NPC], dt.float32, "invdeg")
        wl = [load_const(wl_d[i], list(wl_d[i].shape), dt.bfloat16, f"wl{i}")
              for i in range(3)]
        wr = [load_const(wr_d[i], list(wr_d[i].shape), dt.bfloat16, f"wr{i}")
              for i in range(3)]
        bias = [load_const(b_d[i], [128, 1], dt.float32, f"b{i}") for i in range(3)]
        idx_sb = load_const(idx_d, [128, sum(NRg) // 16], dt.int16, "idx")
        va_sb = load_const(va_d, [128, SNRT * R], dt.bfloat16, "va")

        # iotas
        iota_col = per.tile([128, 1], dt.bfloat16, tag="iotac")
        nc.gpsimd.iota(iota_col[:], pattern=[[1, 1]], base=0,
                       channel_multiplier=1, allow_small_or_imprecise_dtypes=True)
        iota_row = per.tile([128, 128], dt.bfloat16, tag="iotar")
        nc.gpsimd.iota(iota_row[:], pattern=[[1, 128]], base=0,
                       channel_multiplier=0, allow_small_or_imprecise_dtypes=True)
        iota2t_d = None
        zbias = per.tile([128, 1], dt.float32, tag="zbias")
        nc.gpsimd.memset(zbias[:], 0.0)

        idf = per.tile([128, 128], dt.float32, tag="idf")
        nc.vector.tensor_copy(out=idf[:], in_=idbf[:])
        hT = [per.tile([128, NPC], dt.bfloat16, tag=f"hT{i}", name=f"hT{i}")
              for i in range(2)]

        # bootstrap: bf16 cast + AllGather + hT[0]
        nc.gpsimd.dma_start(out=xsb[:, :], in_=x_sl[:, :])
        nc.gpsimd.collective_compute(
            "AllGather", mybir.AluOpType.bypass, replica_groups=groups,
            ins=[xsb.ap().opt()], outs=[hf[0].ap().opt()])
        for w in range(W):
            rows = mpool.tile([128, D_IN], dt.bfloat16, tag="rows_in")
            nc.sync.dma_start(out=rows[:], in_=xsb[w * 128:(w + 1) * 128, :])
            tp = pp_t.tile([128, 128], dt.bfloat16, tag="tp")
            nc.tensor.transpose(out=tp[:], in_=rows[:], identity=idbf[:])
            nc.vector.tensor_copy(out=hT[0][:, w * 128:(w + 1) * 128], in_=tp[:])

        def do_layer(li, hf_in, hT_in, hT_out, hsl_out, hf_out, last):
            dout = D_OUT if last else D_HID
            stag = stag2[li % 2]
            # ---- phase 1: expand into staging ----
            def build_pchunk(pc):
                nmm = min(MMC, n_mm - pc * MMC)
                pvch = ppool.tile([128, nmm * 128], dt.bfloat16,
                                  tag="pvch", name="pvch")
                nc.sync.dma_start(
                    out=pvch[:],
                    in_=pv_d[:1, pc * MMC * 128: (pc * MMC + nmm) * 128]
                    .partition_broadcast(128))
                pstrip = ppool.tile([128, nmm * 128], dt.bfloat16,
                                    tag="pstrip", name="pstrip")
                nc.vector.tensor_tensor(
                    out=pstrip[:],
                    in0=ioct[:, :nmm * 128],
                    in1=pvch[:],
                    op=mybir.AluOpType.is_equal)
                return pstrip

            npc_tot = -(-n_mm // MMC)
            pstrips = {0: build_pchunk(0)}
            cur_xc = -1
            xch = None
            chunkb = None
            ps = None
            for m in range(n_mm):
                b, wsx = mm_block[m], mm_ws[m]
                xc = wsx // CHW
                if xc != cur_xc:
                    cur_xc = xc
                    nw = min(CHW, NWS - xc * CHW)
                    xch = xpool.tile([128, nw, 128], dt.bfloat16, tag="xch",
                                     name="xch")
                    nc.sync.dma_start(
                        out=xch[:, :, :],
                        in_=hf_in[xc * CHW * 128: (xc * CHW + nw) * 128, :]
                        .rearrange("(b p) f -> p b f", p=128))
                pc = m // MMC
                if m % MMC == 0 and pc + 1 < npc_tot:
                    pstrips[pc + 1] = build_pchunk(pc + 1)
                    pstrips.pop(pc - 1, None)
                pstrip = pstrips[pc]
                if mm_start[m]:
                    ps = pp_e.tile([128, 128], dt.float32, tag="exp")
                nc.tensor.matmul(
                    out=ps[:],
                    lhsT=pstrip[:, (m - pc * MMC) * 128:(m - pc * MMC + 1) * 128],
                    rhs=xch[:, wsx - xc * CHW, :],
                    start=mm_start[m], stop=mm_stop[m])
                if mm_stop[m]:
                    bb = b % CHB
                    if bb == 0:
                        nb = min(CHB, NBLK - b)
                        chunkb = cpool.tile([128, nb, 128], dt.bfloat16,
                                            tag="chunk", name="chunk")
                    nc.scalar.activation(
                        out=chunkb[:, bb, :], in_=ps[:],
                        func=mybir.ActivationFunctionType.Identity)
                    if bb == nb - 1 or b == NBLK - 1:
                        b0 = b - bb
                        nc.sync.dma_start(
                            out=stag[b0 * 16:(b0 + bb + 1) * 16, :]
                            .rearrange("(b ph) (pl f) -> (ph pl) b f",
                                       ph=16, pl=8, f=128),
                            in_=chunkb[:, :bb + 1, :])
            # ---- phase 2: collect + aggregate + dense ----
            ixo = 0
            vo = 0
            for g in range(NG):
                nr = NRg[g]
                nt = nrt[g]
                g2 = gpool.tile([128, nt, R * 128], dt.bfloat16, tag="g2",
                                name="g2")
                for k0 in range(0, nr, 512):
                    kn = min(512, nr - k0)
                    nc.gpsimd.dma_gather(
                        g2[:, k0 // 128: (k0 + kn) // 128, :],
                        stag[:, :],
                        idx_sb[:, ixo + k0 // 16: ixo + (k0 + kn) // 16],
                        kn, kn, R * 128)
                ixo += nr // 16
                nw = min(2 * 128, (W - 2 * g) * 128)
                pwin = slice(2 * g * 128, 2 * g * 128 + nw)
                sstrip = spool.tile([128, nt * R, 256], dt.bfloat16,
                                    tag="sstrip", name="sstrip")
                for t in range(nt):
                    nc.vector.tensor_tensor(
                        out=sstrip[:, t * R:(t + 1) * R, :],
                        in0=va_sb[:, vo + t * R: vo + (t + 1) * R]
                        .rearrange("p (j o) -> p j o", o=1)
                        .to_broadcast([128, R, 256]),
                        in1=io2t[:, :].rearrange("p (j c) -> p j c", c=256),
                        op=mybir.AluOpType.is_equal)
                ps2 = pp.tile([128, 256], dt.float32, tag="agg")
                nmm2 = nt * R
                for t in range(nt):
                    for pos in range(R):
                        j = t * R + pos
                        nc.tensor.matmul(
                            out=ps2[:, :nw],
                            lhsT=g2[:, t, pos * 128:(pos + 1) * 128],
                            rhs=sstrip[:, j, :nw],
                            start=(j == 0), stop=(j == nmm2 - 1))
                aggsc = mpool.tile([128, 256], dt.bfloat16, tag="aggsc")
                nc.vector.tensor_tensor(out=aggsc[:, :nw], in0=ps2[:, :nw],
                                        in1=invdeg[:, pwin],
                                        op=mybir.AluOpType.mult)
                pd = pp_d.tile([128, 256], dt.float32, tag="dense")
                nc.tensor.matmul(out=pd[:dout, :nw], lhsT=wl[li][:],
                                 rhs=aggsc[:, :nw], start=True, stop=False)
                nc.tensor.matmul(out=pd[:dout, :nw], lhsT=wr[li][:],
                                 rhs=hT_in[:, pwin], start=False, stop=True)
                if not last:
                    nc.scalar.activation(
                        out=hT_out[:, pwin], in_=pd[:, :nw],
                        func=mybir.ActivationFunctionType.Relu,
                        bias=bias[li][:, :1])
                    for hh in range(nw // 128):
                        w = 2 * g + hh
                        win = slice(w * 128, (w + 1) * 128)
                        tp = pp_t.tile([128, 128], dt.bfloat16, tag="tp")
                        nc.tensor.transpose(out=tp[:], in_=hT_out[:, win],
                                            identity=idbf[:])
                        rows = mpool.tile([128, D_IN], dt.bfloat16,
                                          tag="rows_out")
                        nc.vector.tensor_copy(out=rows[:], in_=tp[:])
                        nc.sync.dma_start(out=hsl_out[win, :], in_=rows[:])
                else:
                    oT = mpool.tile([128, 256], dt.float32, tag="oT")
                    nc.scalar.activation(
                        out=oT[:dout, :nw], in_=pd[:dout, :nw],
                        func=mybir.ActivationFunctionType.Identity,
                        bias=bias[li][:dout, :1])
                    if dout < 128:
                        nc.vector.memset(oT[dout:, :nw], 0.0)
                    for hh in range(nw // 128):
                        w = 2 * g + hh
                        win = slice(w * 128, (w + 1) * 128)
                        tp = pp_t.tile([128, 128], dt.float32, tag="tpf")
                        nc.tensor.transpose(out=tp[:], in_=oT[:, hh * 128:(hh + 1) * 128],
                                            identity=idf[:])
                        negmax = mpool.tile([128, 1], dt.float32, tag="negmax")
                        nc.vector.tensor_reduce(out=negmax[:], in_=tp[:, :D_OUT],
                                                axis=mybir.AxisListType.X,
                                                op=mybir.AluOpType.max, negate=True)
                        esb = mpool.tile([128, D_OUT], dt.float32, tag="esb")
                        nc.scalar.activation(out=esb[:], in_=tp[:, :D_OUT],
                                             func=mybir.ActivationFunctionType.Exp,
                                             bias=negmax[:, :1])
                        ssum = mpool.tile([128, 1], dt.float32, tag="ssum")
                        nc.vector.tensor_reduce(out=ssum[:], in_=esb[:],
                                                axis=mybir.AxisListType.X,
                                                op=mybir.AluOpType.add)
                        lns = mpool.tile([128, 1], dt.float32, tag="lns")
                        nc.scalar.activation(out=lns[:], in_=ssum[:],
                                             func=mybir.ActivationFunctionType.Ln)
                        csub = mpool.tile([128, 1], dt.float32, tag="csub")
                        nc.vector.tensor_tensor(out=csub[:], in0=lns[:], in1=negmax[:],
                                                op=mybir.AluOpType.subtract)
                        res = mpool.tile([128, D_OUT], dt.float32, tag="res")
                        nc.vector.tensor_tensor(out=res[:], in0=tp[:, :D_OUT],
                                                in1=csub[:, :1].to_broadcast([128, D_OUT]),
                                                op=mybir.AluOpType.subtract)
                        nc.sync.dma_start(out=out_d[win, :], in_=res[:])
                vo += nt * R
            if not last:
                nc.gpsimd.collective_compute(
                    "AllGather", mybir.AluOpType.bypass, replica_groups=groups,
                    ins=[hsl_out.ap().opt()], outs=[hf_out.ap().opt()])

        do_layer(0, hf[0], hT[0], hT[1], hsl[0], hf[1], last=False)
        do_layer(1, hf[1], hT[1], hT[0], hsl[1], hf[2], last=False)
        do_layer(2, hf[2], hT[0], None, None, None, last=True)

    nc.compile()
    return nc


def _make_inputs(plan, x, W1_l, W1_r, b1, Wm_l, Wm_r, bm, W2_l, W2_r, b2):
    ident = np.eye(128, dtype=np.float32).astype(BF16)

    def pad_bias(b):
        v = np.zeros((128, 1), np.float32)
        v[: len(b), 0] = np.asarray(b, np.float32)
        return v

    common = {
        "idbf": ident,
        "ioct": np.tile(np.arange(128, dtype=np.float32)[:, None],
                        (1, MMC * 128)).astype(BF16),
        "io2t": np.tile(np.tile(np.arange(256, dtype=np.float32), R)[None, :],
                        (128, 1)).astype(BF16),
        "wl0": np.asarray(W1_l, np.float32).astype(BF16),
        "wr0": np.asarray(W1_r, np.float32).astype(BF16),
        "b0": pad_bias(b1),
        "wl1": np.asarray(Wm_l, np.float32).astype(BF16),
        "wr1": np.asarray(Wm_r, np.float32).astype(BF16),
        "b1": pad_bias(bm),
        "wl2": np.asarray(W2_l, np.float32).astype(BF16),
        "wr2": np.asarray(W2_r, np.float32).astype(BF16),
        "b2": pad_bias(b2),
    }
    x = np.asarray(x, np.float32)
    in_maps = []
    for c in range(N_CORES):
        xs = np.zeros((NPC, D_IN), np.float32)
        xs[:NPC_RAW] = x[c * NPC_RAW:(c + 1) * NPC_RAW]
        m = dict(common)
        m["x_slice"] = xs
        m["invdeg"] = plan["invdeg"][c]
        m["pv"] = plan["PV"][c]
        m["idx"] = np.tile(np.concatenate(
            [plan["idx_w"][g][c] for g in range(NG)], axis=1), (8, 1))
        m["va"] = np.concatenate([plan["VA"][g][c] for g in range(NG)], axis=1)
        in_maps.append(m)
    return in_maps


def _postprocess(results):
    out = np.empty((N, D_OUT), np.float32)
    for c in range(N_CORES):
        out[c * NPC_RAW:(c + 1) * NPC_RAW] = results[c]["out"][:NPC_RAW]
    return out


_CACHE = {}


def kernel(x, edge_index, W1_l, W1_r, b1, Wm_l, Wm_r, bm, W2_l, W2_r, b2,
           _trace=False):
    from concourse.bass_utils import run_bass_kernel_spmd

    edge_index = np.asarray(edge_index)
    key = hash(edge_index.tobytes())
    if key not in _CACHE:
        plan = _plan(edge_index)
        nc = _build(plan)
        _CACHE[key] = (plan, nc)
    plan, nc = _CACHE[key]
    in_maps = _make_inputs(plan, x, W1_l, W1_r, b1, Wm_l, Wm_r, bm,
                           W2_l, W2_r, b2)
    res = run_bass_kernel_spmd(nc, in_maps, core_ids=list(range(N_CORES)),
                               trace=_trace)
    out = _postprocess(res.results)
    if _trace:
        kernel._last_exec_ns = res.exec_time_ns
        kernel._last_res = res
    return out


# revision 11
# speedup vs baseline: 1.0295x; 1.0043x over previous
"""3-layer GraphSAGE (mean agg) on 8 Trainium2 NeuronCores — two-phase radix.

The old per-edge dma_gather spent ~9ns/edge of GPSIMD(Q7) descriptor
generation (3.1ms of a 3.6ms kernel).  Measured on HW: descriptor cost is
per-INDEX, independent of element size.  So:
  Phase 1 (expand): stream the replicated node table sequentially; for each
    128-row table window, a PE one-hot matmul scatters its outgoing edge rows
    into a dst-ordered slot array (8-slot runs, each run pure to one
    dst-window-pair); slots stream to a DRAM staging buffer via large DMAs.
  Phase 2 (collect): per dst-window-pair, dma_gather with elem_size=2KB pulls
    8 edge-rows per descriptor (8x fewer Q7 descriptors); PE aggregates via
    8-position one-hot matmuls into per-window PSUM tiles; then the dense
    SAGE matmuls / ReLU / log_softmax tail as before.
Weights replicated; node features exchanged between layers via AllGather.
"""

import sys
import numpy as np

for _p in ("/opt/trn_rl_repo", "/root/.axon_site/_ro/trn_rl_repo"):
    if _p not in sys.path:
        sys.path.append(_p)

import ml_dtypes

BF16 = ml_dtypes.bfloat16

N = 50000
E = 800000
D_IN = 128
D_HID = 128
D_OUT = 64
N_CORES = 8
WIN = 128
NPC_RAW = N // N_CORES            # 6250
NPC = ((NPC_RAW + WIN - 1) // WIN) * WIN   # 6272
W = NPC // WIN                    # 49
NG = (W + 1) // 2                 # 25 dst-window groups (pairs, last single)
HTAB = N_CORES * NPC              # 50176
NWS = HTAB // WIN                 # 392 table windows
R = 8                             # slots per run (2KB bf16)
CHW = 32                          # table windows per phase-1 chunk
CHB = 16                          # blocks per staging chunk
MMC = 32                          # matmuls per P-strip build


def _plan(edge_index):
    src = np.asarray(edge_index[0], dtype=np.int64)
    dst = np.asarray(edge_index[1], dtype=np.int64)
    ec = dst // NPC_RAW
    rank = dst % NPC_RAW
    ew = rank // WIN
    dl = rank % WIN
    grp = ew // 2
    h = ew % 2
    ep = (src // NPC_RAW) * NPC + (src % NPC_RAW)
    ws = ep // WIN
    sl = ep % WIN

    cnt = np.zeros((N_CORES, NWS, NG), np.int64)
    np.add.at(cnt, (ec, ws, grp), 1)
    runs_cell = -(-cnt // R)                      # [8, NWS, NG]
    # slots per (core, ws): run-aligned
    s_c_ws = runs_cell.sum(axis=2) * R            # [8, NWS]
    REG = s_c_ws.max(axis=0)                      # [NWS] (multiple of 8)
    RB = np.concatenate([[0], np.cumsum(REG)])    # region base slots
    LSLOT = int(RB[-1])
    NBLK = -(-LSLOT // 128)
    LSLOT_PAD = NBLK * 128
    NRUN = LSLOT_PAD // R
    assert NRUN < 32768, f"run index overflow: {NRUN}"

    # per-core cell base slot
    cum_runs = np.cumsum(runs_cell, axis=2) - runs_cell    # exclusive, [8,NWS,NG]
    cb = RB[None, :NWS, None] + cum_runs * R               # [8, NWS, NG]

    # edge slot assignment
    okey = np.lexsort((dl, grp, ws, ec))
    ec_s, ws_s, grp_s, sl_s, dl_s, h_s = (a[okey] for a in (ec, ws, grp, sl, dl, h))
    cell_id = (ec_s * NWS + ws_s) * NG + grp_s
    newcell = np.concatenate([[True], cell_id[1:] != cell_id[:-1]])
    gstart = np.maximum.accumulate(np.where(newcell, np.arange(E), 0))
    krank = np.arange(E) - gstart
    slot = cb[ec_s, ws_s, grp_s] + krank

    srcl = np.full((N_CORES, LSLOT_PAD), -1.0, np.float32)
    dA = np.full((N_CORES, LSLOT_PAD), -1.0, np.float32)
    srcl[ec_s, slot] = sl_s
    dA[ec_s, slot] = dl_s + 128.0 * h_s

    # matmul descriptor list: blocks split into region segments
    reg_end = RB[1:]
    mm_block, mm_ws, mm_start, mm_stop, mm_lo, mm_hi = [], [], [], [], [], []
    for b in range(NBLK):
        lo, hi = b * 128, (b + 1) * 128
        wlo = int(np.searchsorted(reg_end, lo, side="right"))
        segs = []
        cur = lo
        wcur = min(wlo, NWS - 1)
        while cur < hi:
            seg_end = min(hi, int(reg_end[wcur]) if wcur < NWS else hi)
            if seg_end <= cur:       # dead tail past LSLOT
                seg_end = hi
            segs.append((wcur, cur, seg_end))
            cur = seg_end
            wcur = min(wcur + 1, NWS - 1)
        for i, (wsx, lo2, hi2) in enumerate(segs):
            mm_block.append(b)
            mm_ws.append(wsx)
            mm_start.append(i == 0)
            mm_stop.append(i == len(segs) - 1)
            mm_lo.append(lo2 - lo)
            mm_hi.append(hi2 - lo)
    n_mm = len(mm_block)

    PV = np.full((N_CORES, n_mm * 128), -1.0, np.float32)
    for m in range(n_mm):
        b, lo2, hi2 = mm_block[m], mm_lo[m], mm_hi[m]
        PV[:, m * 128 + lo2: m * 128 + hi2] = \
            srcl[:, b * 128 + lo2: b * 128 + hi2]

    # phase-2 run lists per group
    NRg, nrt, idx_w, VA = [], [], [], []
    for g in range(NG):
        rls = []
        for c in range(N_CORES):
            parts = []
            for wsx in range(NWS):
                k = int(runs_cell[c, wsx, g])
                if k:
                    base = int(cb[c, wsx, g]) // R
                    parts.append(base + np.arange(k))
            rl = np.concatenate(parts) if parts else np.zeros(0, np.int64)
            rls.append(rl)
        mx = max(len(r) for r in rls)
        nr = ((mx + 127) // 128) * 128
        NRg.append(nr)
        nrt.append(nr // 128)
        iw, va = [], []
        for c in range(N_CORES):
            rl = np.zeros(nr, np.int64)
            rl[: len(rls[c])] = rls[c]
            # wrap for dma_gather: per 512-call columns of reshape(-1,16).T
            cols = []
            for b0 in range(0, nr, 512):
                blk = rl[b0: b0 + 512]
                cols.append(blk.reshape(-1, 16).T)
            iw.append(np.concatenate(cols, axis=1).astype(np.int16))
            a = dA[c][(rl[:, None] * R + np.arange(R))].astype(np.float32)
            if len(rls[c]) < nr:    # padded runs contribute nothing
                a[len(rls[c]):] = -1.0
            va.append(a.reshape(nr // 128, 128, R).transpose(1, 0, 2)
                      .reshape(128, -1).astype(BF16))
        idx_w.append(iw)
        VA.append(va)

    deg = np.bincount(dst, minlength=N).astype(np.float32)
    invdeg = []
    for c in range(N_CORES):
        v = np.ones(NPC, np.float32)
        v[:NPC_RAW] = 1.0 / np.maximum(deg[c * NPC_RAW:(c + 1) * NPC_RAW], 1.0)
        invdeg.append(np.tile(v[None, :], (WIN, 1)))

    return {
        "LSLOT_PAD": LSLOT_PAD, "NBLK": NBLK, "NRUN": NRUN, "n_mm": n_mm,
        "mm_block": mm_block, "mm_ws": mm_ws, "mm_start": mm_start,
        "mm_stop": mm_stop, "NRg": NRg, "nrt": nrt,
        "PV": [np.ascontiguousarray(PV[c]).astype(BF16)[None, :] for c in range(N_CORES)],
        "idx_w": idx_w, "VA": VA, "invdeg": invdeg,
    }


def _build(plan):
    import concourse.bacc as bacc
    import concourse.bass as bass
    import concourse.mybir as mybir
    import concourse.tile as tile
    from contextlib import ExitStack

    dt = mybir.dt
    NBLK, NRUN, n_mm = plan["NBLK"], plan["NRUN"], plan["n_mm"]
    NRg, nrt = plan["NRg"], plan["nrt"]
    mm_block, mm_ws = plan["mm_block"], plan["mm_ws"]
    mm_start, mm_stop = plan["mm_start"], plan["mm_stop"]
    SNRT = sum(nrt)

    nc = bacc.Bacc("TRN2", target_bir_lowering=False)

    x_sl = nc.dram_tensor("x_slice", [NPC, D_IN], dt.float32, kind="ExternalInput")
    pv_d = nc.dram_tensor("pv", [1, n_mm * 128], dt.bfloat16, kind="ExternalInput")
    idx_d = nc.dram_tensor("idx", [128, (sum(NRg)) // 16], dt.int16,
                           kind="ExternalInput")
    va_d = nc.dram_tensor("va", [128, SNRT * R], dt.bfloat16, kind="ExternalInput")
    invdeg_d = nc.dram_tensor("invdeg", [128, NPC], dt.float32, kind="ExternalInput")
    idbf_d = nc.dram_tensor("idbf", [128, 128], dt.bfloat16, kind="ExternalInput")
    ioct_d = nc.dram_tensor("ioct", [128, MMC * 128], dt.bfloat16,
                            kind="ExternalInput")
    io2t_d = nc.dram_tensor("io2t", [128, R * 256], dt.bfloat16,
                            kind="ExternalInput")
    wl_d, wr_d, b_d = [], [], []
    for li, (din, dout) in enumerate([(D_IN, D_HID), (D_HID, D_HID),
                                      (D_HID, D_OUT)]):
        wl_d.append(nc.dram_tensor(f"wl{li}", [din, dout], dt.bfloat16,
                                   kind="ExternalInput"))
        wr_d.append(nc.dram_tensor(f"wr{li}", [din, dout], dt.bfloat16,
                                   kind="ExternalInput"))
        b_d.append(nc.dram_tensor(f"b{li}", [128, 1], dt.float32,
                                  kind="ExternalInput"))

    xsb = nc.dram_tensor("xsb", [NPC, D_IN], dt.bfloat16)
    hsl = [nc.dram_tensor(f"hsl{i}", [NPC, D_IN], dt.bfloat16) for i in range(2)]
    hf = [nc.dram_tensor(f"hf{i}", [HTAB, D_IN], dt.bfloat16) for i in range(3)]
    stag2 = [nc.dram_tensor(f"stag{i}", [NRUN, R * 128], dt.bfloat16)
             for i in range(2)]
    out_d = nc.dram_tensor("out", [NPC, D_OUT], dt.float32, kind="ExternalOutput")

    groups = [list(range(N_CORES))]

    with tile.TileContext(nc) as tc, ExitStack() as ctx:
        per = ctx.enter_context(tc.tile_pool(name="persist", bufs=1))
        xpool = ctx.enter_context(tc.tile_pool(name="xch", bufs=2))
        ppool = ctx.enter_context(tc.tile_pool(name="pstrip", bufs=2))
        cpool = ctx.enter_context(tc.tile_pool(name="chunk", bufs=3))
        gpool = ctx.enter_context(tc.tile_pool(name="g2", bufs=2))
        spool = ctx.enter_context(tc.tile_pool(name="sstrip", bufs=2))
        mpool = ctx.enter_context(tc.tile_pool(name="misc", bufs=2))
        pp_e = ctx.enter_context(tc.tile_pool(name="ps_exp", bufs=2, space="PSUM"))
        pp = ctx.enter_context(tc.tile_pool(name="ps_agg", bufs=2, space="PSUM"))
        pp_d = ctx.enter_context(tc.tile_pool(name="ps_dense", bufs=2, space="PSUM"))
        pp_t = ctx.enter_context(tc.tile_pool(name="ps_tr", bufs=1, space="PSUM"))

        def load_const(dram, shape, dtp, tag):
            t = per.tile(shape, dtp, tag=tag, name=tag)
            nc.sync.dma_start(out=t[:], in_=dram[:])
            return t

        idbf = load_const(idbf_d, [128, 128], dt.bfloat16, "idbf")
        ioct = load_const(ioct_d, [128, MMC * 128], dt.bfloat16, "ioct")
        io2t = load_const(io2t_d, [128, R * 256], dt.bfloat16, "io2t")
        invdeg = load_const(invdeg_d, [128, NPC], dt.float32, "invdeg")
        wl = [load_const(wl_d[i], list(wl_d[i].shape), dt.bfloat16, f"wl{i}")
              for i in range(3)]
        wr = [load_const(wr_d[i], list(wr_d[i].shape), dt.bfloat16, f"wr{i}")
              for i in range(3)]
        bias = [load_const(b_d[i], [128, 1], dt.float32, f"b{i}") for i in range(3)]
        idx_sb = load_const(idx_d, [128, sum(NRg) // 16], dt.int16, "idx")
        va_sb = load_const(va_d, [128, SNRT * R], dt.bfloat16, "va")

        # iotas
        iota_col = per.tile([128, 1], dt.bfloat16, tag="iotac")
        nc.gpsimd.iota(iota_col[:], pattern=[[1, 1]], base=0,
                       channel_multiplier=1, allow_small_or_imprecise_dtypes=True)
        iota_row = per.tile([128, 128], dt.bfloat16, tag="iotar")
        nc.gpsimd.iota(iota_row[:], pattern=[[1, 128]], base=0,
                       channel_multiplier=0, allow_small_or_imprecise_dtypes=True)
        iota2t_d = None
        zbias = per.tile([128, 1], dt.float32, tag="zbias")
        nc.gpsimd.memset(zbias[:], 0.0)

        idf = per.tile([128, 128], dt.float32, tag="idf")
        nc.vector.tensor_copy(out=idf[:], in_=idbf[:])
        hT = [per.tile([128, NPC], dt.bfloat16, tag=f"hT{i}", name=f"hT{i}")
              for i in range(2)]

        # bootstrap: bf16 cast + AllGather + hT[0]
        nc.gpsimd.dma_start(out=xsb[:, :], in_=x_sl[:, :])
        nc.gpsimd.collective_compute(
            "AllGather", mybir.AluOpType.bypass, replica_groups=groups,
            ins=[xsb.ap().opt()], outs=[hf[0].ap().opt()])
        for w in range(W):
            rows = mpool.tile([128, D_IN], dt.bfloat16, tag="rows_in")
            nc.sync.dma_start(out=rows[:], in_=xsb[w * 128:(w + 1) * 128, :])
            tp = pp_t.tile([128, 128], dt.bfloat16, tag="tp")
            nc.tensor.transpose(out=tp[:], in_=rows[:], identity=idbf[:])
            nc.vector.tensor_copy(out=hT[0][:, w * 128:(w + 1) * 128], in_=tp[:])

        def do_layer(li, hf_in, hT_in, hT_out, hsl_out, hf_out, last):
            dout = D_OUT if last else D_HID
            stag = stag2[li % 2]
            # ---- phase 1: expand into staging ----
            def build_pchunk(pc):
                nmm = min(MMC, n_mm - pc * MMC)
                pvch = ppool.tile([128, nmm * 128], dt.bfloat16,
                                  tag="pvch", name="pvch")
                nc.sync.dma_start(
                    out=pvch[:],
                    in_=pv_d[:1, pc * MMC * 128: (pc * MMC + nmm) * 128]
                    .partition_broadcast(128))
                pstrip = ppool.tile([128, nmm * 128], dt.bfloat16,
                                    tag="pstrip", name="pstrip")
                nc.vector.tensor_tensor(
                    out=pstrip[:],
                    in0=ioct[:, :nmm * 128],
                    in1=pvch[:],
                    op=mybir.AluOpType.is_equal)
                return pstrip

            def load_xchunk(xc):
                nw = min(CHW, NWS - xc * CHW)
                xt = xpool.tile([128, nw, 128], dt.bfloat16, tag="xch",
                                name="xch")
                nc.sync.dma_start(
                    out=xt[:, :, :],
                    in_=hf_in[xc * CHW * 128: (xc * CHW + nw) * 128, :]
                    .rearrange("(b p) f -> p b f", p=128))
                return xt

            nxc_tot = -(-NWS // CHW)
            npc_tot = -(-n_mm // MMC)
            xchs = {0: load_xchunk(0)}
            pstrips = {0: build_pchunk(0)}
            cur_xc = 0
            chunkb = None
            ps = None
            for m in range(n_mm):
                b, wsx = mm_block[m], mm_ws[m]
                xc = wsx // CHW
                if xc != cur_xc or m == 0:
                    cur_xc = xc
                    if xc + 1 < nxc_tot and xc + 1 not in xchs:
                        xchs[xc + 1] = load_xchunk(xc + 1)
                    xchs.pop(xc - 1, None)
                xch = xchs[xc]
                pc = m // MMC
                if m % MMC == 0 and pc + 1 < npc_tot:
                    pstrips[pc + 1] = build_pchunk(pc + 1)
                    pstrips.pop(pc - 1, None)
                pstrip = pstrips[pc]
                if mm_start[m]:
                    ps = pp_e.tile([128, 128], dt.float32, tag="exp")
                nc.tensor.matmul(
                    out=ps[:],
                    lhsT=pstrip[:, (m - pc * MMC) * 128:(m - pc * MMC + 1) * 128],
                    rhs=xch[:, wsx - xc * CHW, :],
                    start=mm_start[m], stop=mm_stop[m])
                if mm_stop[m]:
                    bb = b % CHB
                    if bb == 0:
                        nb = min(CHB, NBLK - b)
                        chunkb = cpool.tile([128, nb, 128], dt.bfloat16,
                                            tag="chunk", name="chunk")
                    nc.scalar.activation(
                        out=chunkb[:, bb, :], in_=ps[:],
                        func=mybir.ActivationFunctionType.Identity)
                    if bb == nb - 1 or b == NBLK - 1:
                        b0 = b - bb
                        nc.sync.dma_start(
                            out=stag[b0 * 16:(b0 + bb + 1) * 16, :]
                            .rearrange("(b ph) (pl f) -> (ph pl) b f",
                                       ph=16, pl=8, f=128),
                            in_=chunkb[:, :bb + 1, :])
            # ---- phase 2: collect + aggregate + dense ----
            ixo = 0
            vo = 0
            for g in range(NG):
                nr = NRg[g]
                nt = nrt[g]
                g2 = gpool.tile([128, nt, R * 128], dt.bfloat16, tag="g2",
                                name="g2")
                for k0 in range(0, nr, 512):
                    kn = min(512, nr - k0)
                    nc.gpsimd.dma_gather(
                        g2[:, k0 // 128: (k0 + kn) // 128, :],
                        stag[:, :],
                        idx_sb[:, ixo + k0 // 16: ixo + (k0 + kn) // 16],
                        kn, kn, R * 128)
                ixo += nr // 16
                nw = min(2 * 128, (W - 2 * g) * 128)
                pwin = slice(2 * g * 128, 2 * g * 128 + nw)
                sstrip = spool.tile([128, nt * R, 256], dt.bfloat16,
                                    tag="sstrip", name="sstrip")
                for t in range(nt):
                    nc.vector.tensor_tensor(
                        out=sstrip[:, t * R:(t + 1) * R, :],
                        in0=va_sb[:, vo + t * R: vo + (t + 1) * R]
                        .rearrange("p (j o) -> p j o", o=1)
                        .to_broadcast([128, R, 256]),
                        in1=io2t[:, :].rearrange("p (j c) -> p j c", c=256),
                        op=mybir.AluOpType.is_equal)
                ps2 = pp.tile([128, 256], dt.float32, tag="agg")
                nmm2 = nt * R
                for t in range(nt):
                    for pos in range(R):
                        j = t * R + pos
                        nc.tensor.matmul(
                            out=ps2[:, :nw],
                            lhsT=g2[:, t, pos * 128:(pos + 1) * 128],
                            rhs=sstrip[:, j, :nw],
                            start=(j == 0), stop=(j == nmm2 - 1))
                aggsc = mpool.tile([128, 256], dt.bfloat16, tag="aggsc")
                nc.vector.tensor_tensor(out=aggsc[:, :nw], in0=ps2[:, :nw],
                                        in1=invdeg[:, pwin],
                                        op=mybir.AluOpType.mult)
                pd = pp_d.tile([128, 256], dt.float32, tag="dense")
                nc.tensor.matmul(out=pd[:dout, :nw], lhsT=wl[li][:],
                                 rhs=aggsc[:, :nw], start=True, stop=False)
                nc.tensor.matmul(out=pd[:dout, :nw], lhsT=wr[li][:],
                                 rhs=hT_in[:, pwin], start=False, stop=True)
                if not last:
                    nc.scalar.activation(
                        out=hT_out[:, pwin], in_=pd[:, :nw],
                        func=mybir.ActivationFunctionType.Relu,
                        bias=bias[li][:, :1])
                    for hh in range(nw // 128):
                        w = 2 * g + hh
                        win = slice(w * 128, (w + 1) * 128)
                        tp = pp_t.tile([128, 128], dt.bfloat16, tag="tp")
                        nc.tensor.transpose(out=tp[:], in_=hT_out[:, win],
                                            identity=idbf[:])
                        rows = mpool.tile([128, D_IN], dt.bfloat16,
                                          tag="rows_out")
                        nc.vector.tensor_copy(out=rows[:], in_=tp[:])
                        nc.sync.dma_start(out=hsl_out[win, :], in_=rows[:])
                else:
                    oT = mpool.tile([128, 256], dt.float32, tag="oT")
                    nc.scalar.activation(
                        out=oT[:dout, :nw], in_=pd[:dout, :nw],
                        func=mybir.ActivationFunctionType.Identity,
                        bias=bias[li][:dout, :1])
                    if dout < 128:
                        nc.vector.memset(oT[dout:, :nw], 0.0)
                    for hh in range(nw // 128):
                        w = 2 * g + hh
                        win = slice(w * 128, (w + 1) * 128)
                        tp = pp_t.tile([128, 128], dt.float32, tag="tpf")
                        nc.tensor.transpose(out=tp[:], in_=oT[:, hh * 128:(hh + 1) * 128],
                                            identity=idf[:])
                        negmax = mpool.tile([128, 1], dt.float32, tag="negmax")
                        nc.vector.tensor_reduce(out=negmax[:], in_=tp[:, :D_OUT],
                                                axis=mybir.AxisListType.X,
                                                op=mybir.AluOpType.max, negate=True)
                        esb = mpool.tile([128, D_OUT], dt.float32, tag="esb")
                        nc.scalar.activation(out=esb[:], in_=tp[:, :D_OUT],
                                             func=mybir.ActivationFunctionType.Exp,
                                             bias=negmax[:, :1])
                        ssum = mpool.tile([128, 1], dt.float32, tag="ssum")
                        nc.vector.tensor_reduce(out=ssum[:], in_=esb[:],
                                                axis=mybir.AxisListType.X,
                                                op=mybir.AluOpType.add)
                        lns = mpool.tile([128, 1], dt.float32, tag="lns")
                        nc.scalar.activation(out=lns[:], in_=ssum[:],
                                             func=mybir.ActivationFunctionType.Ln)
                        csub = mpool.tile([128, 1], dt.float32, tag="csub")
                        nc.vector.tensor_tensor(out=csub[:], in0=lns[:], in1=negmax[:],
                                                op=mybir.AluOpType.subtract)
                        res = mpool.tile([128, D_OUT], dt.float32, tag="res")
                        nc.vector.tensor_tensor(out=res[:], in0=tp[:, :D_OUT],
                                                in1=csub[:, :1].to_broadcast([128, D_OUT]),
                                                op=mybir.AluOpType.subtract)
                        nc.sync.dma_start(out=out_d[win, :], in_=res[:])
                vo += nt * R
            if not last:
                nc.gpsimd.collective_compute(
                    "AllGather", mybir.AluOpType.bypass, replica_groups=groups,
                    ins=[hsl_out.ap().opt()], outs=[hf_out.ap().opt()])

        do_layer(0, hf[0], hT[0], hT[1], hsl[0], hf[1], last=False)
        do_layer(1, hf[1], hT[1], hT[0], hsl[1], hf[2], last=False)
        do_layer(2, hf[2], hT[0], None, None, None, last=True)

    nc.compile()
    return nc


def _make_inputs(plan, x, W1_l, W1_r, b1, Wm_l, Wm_r, bm, W2_l, W2_r, b2):
    ident = np.eye(128, dtype=np.float32).astype(BF16)

    def pad_bias(b):
        v = np.zeros((128, 1), np.float32)
        v[: len(b), 0] = np.asarray(b, np.float32)
        return v

    common = {
        "idbf": ident,
        "ioct": np.tile(np.arange(128, dtype=np.float32)[:, None],
                        (1, MMC * 128)).astype(BF16),
        "io2t": np.tile(np.tile(np.arange(256, dtype=np.float32), R)[None, :],
                        (128, 1)).astype(BF16),
        "wl0": np.asarray(W1_l, np.float32).astype(BF16),
        "wr0": np.asarray(W1_r, np.float32).astype(BF16),
        "b0": pad_bias(b1),
        "wl1": np.asarray(Wm_l, np.float32).astype(BF16),
        "wr1": np.asarray(Wm_r, np.float32).astype(BF16),
        "b1": pad_bias(bm),
        "wl2": np.asarray(W2_l, np.float32).astype(BF16),
        "wr2": np.asarray(W2_r, np.float32).astype(BF16),
        "b2": pad_bias(b2),
    }
    x = np.asarray(x, np.float32)
    in_maps = []
    for c in range(N_CORES):
        xs = np.zeros((NPC, D_IN), np.float32)
        xs[:NPC_RAW] = x[c * NPC_RAW:(c + 1) * NPC_RAW]
        m = dict(common)
        m["x_slice"] = xs
        m["invdeg"] = plan["invdeg"][c]
        m["pv"] = plan["PV"][c]
        m["idx"] = np.tile(np.concatenate(
            [plan["idx_w"][g][c] for g in range(NG)], axis=1), (8, 1))
        m["va"] = np.concatenate([plan["VA"][g][c] for g in range(NG)], axis=1)
        in_maps.append(m)
    return in_maps


def _postprocess(results):
    out = np.empty((N, D_OUT), np.float32)
    for c in range(N_CORES):
        out[c * NPC_RAW:(c + 1) * NPC_RAW] = results[c]["out"][:NPC_RAW]
    return out


_CACHE = {}


def kernel(x, edge_index, W1_l, W1_r, b1, Wm_l, Wm_r, bm, W2_l, W2_r, b2,
           _trace=False):
    from concourse.bass_utils import run_bass_kernel_spmd

    edge_index = np.asarray(edge_index)
    key = hash(edge_index.tobytes())
    if key not in _CACHE:
        plan = _plan(edge_index)
        nc = _build(plan)
        _CACHE[key] = (plan, nc)
    plan, nc = _CACHE[key]
    in_maps = _make_inputs(plan, x, W1_l, W1_r, b1, Wm_l, Wm_r, bm,
                           W2_l, W2_r, b2)
    res = run_bass_kernel_spmd(nc, in_maps, core_ids=list(range(N_CORES)),
                               trace=_trace)
    out = _postprocess(res.results)
    if _trace:
        kernel._last_exec_ns = res.exec_time_ns
        kernel._last_res = res
    return out


# revision 12
# speedup vs baseline: 1.0515x; 1.0213x over previous
"""3-layer GraphSAGE (mean agg) on 8 Trainium2 NeuronCores — two-phase radix.

The old per-edge dma_gather spent ~9ns/edge of GPSIMD(Q7) descriptor
generation (3.1ms of a 3.6ms kernel).  Measured on HW: descriptor cost is
per-INDEX, independent of element size.  So:
  Phase 1 (expand): stream the replicated node table sequentially; for each
    128-row table window, a PE one-hot matmul scatters its outgoing edge rows
    into a dst-ordered slot array (8-slot runs, each run pure to one
    dst-window-pair); slots stream to a DRAM staging buffer via large DMAs.
  Phase 2 (collect): per dst-window-pair, dma_gather with elem_size=2KB pulls
    8 edge-rows per descriptor (8x fewer Q7 descriptors); PE aggregates via
    8-position one-hot matmuls into per-window PSUM tiles; then the dense
    SAGE matmuls / ReLU / log_softmax tail as before.
Weights replicated; node features exchanged between layers via AllGather.
"""

import sys
import numpy as np

for _p in ("/opt/trn_rl_repo", "/root/.axon_site/_ro/trn_rl_repo"):
    if _p not in sys.path:
        sys.path.append(_p)

import ml_dtypes

BF16 = ml_dtypes.bfloat16

N = 50000
E = 800000
D_IN = 128
D_HID = 128
D_OUT = 64
N_CORES = 8
WIN = 128
NPC_RAW = N // N_CORES            # 6250
NPC = ((NPC_RAW + WIN - 1) // WIN) * WIN   # 6272
W = NPC // WIN                    # 49
NG = (W + 1) // 2                 # 25 dst-window groups (pairs, last single)
HTAB = N_CORES * NPC              # 50176
NWS = HTAB // WIN                 # 392 table windows
R = 8                             # slots per run (2KB bf16)
CHW = 32                          # table windows per phase-1 chunk
CHB = 24                          # blocks per staging chunk
MMC = 32                          # matmuls per P-strip build


def _plan(edge_index):
    src = np.asarray(edge_index[0], dtype=np.int64)
    dst = np.asarray(edge_index[1], dtype=np.int64)
    ec = dst // NPC_RAW
    rank = dst % NPC_RAW
    ew = rank // WIN
    dl = rank % WIN
    grp = ew // 2
    h = ew % 2
    ep = (src // NPC_RAW) * NPC + (src % NPC_RAW)
    ws = ep // WIN
    sl = ep % WIN

    cnt = np.zeros((N_CORES, NWS, NG), np.int64)
    np.add.at(cnt, (ec, ws, grp), 1)
    runs_cell = -(-cnt // R)                      # [8, NWS, NG]
    # slots per (core, ws): run-aligned
    s_c_ws = runs_cell.sum(axis=2) * R            # [8, NWS]
    REG = s_c_ws.max(axis=0)                      # [NWS] (multiple of 8)
    RB = np.concatenate([[0], np.cumsum(REG)])    # region base slots
    LSLOT = int(RB[-1])
    NBLK = -(-LSLOT // 128)
    LSLOT_PAD = NBLK * 128
    NRUN = LSLOT_PAD // R
    assert NRUN < 32768, f"run index overflow: {NRUN}"

    # per-core cell base slot
    cum_runs = np.cumsum(runs_cell, axis=2) - runs_cell    # exclusive, [8,NWS,NG]
    cb = RB[None, :NWS, None] + cum_runs * R               # [8, NWS, NG]

    # edge slot assignment
    okey = np.lexsort((dl, grp, ws, ec))
    ec_s, ws_s, grp_s, sl_s, dl_s, h_s = (a[okey] for a in (ec, ws, grp, sl, dl, h))
    cell_id = (ec_s * NWS + ws_s) * NG + grp_s
    newcell = np.concatenate([[True], cell_id[1:] != cell_id[:-1]])
    gstart = np.maximum.accumulate(np.where(newcell, np.arange(E), 0))
    krank = np.arange(E) - gstart
    slot = cb[ec_s, ws_s, grp_s] + krank

    srcl = np.full((N_CORES, LSLOT_PAD), -1.0, np.float32)
    dA = np.full((N_CORES, LSLOT_PAD), -1.0, np.float32)
    srcl[ec_s, slot] = sl_s
    dA[ec_s, slot] = dl_s + 128.0 * h_s

    # matmul descriptor list: blocks split into region segments
    reg_end = RB[1:]
    mm_block, mm_ws, mm_start, mm_stop, mm_lo, mm_hi = [], [], [], [], [], []
    for b in range(NBLK):
        lo, hi = b * 128, (b + 1) * 128
        wlo = int(np.searchsorted(reg_end, lo, side="right"))
        segs = []
        cur = lo
        wcur = min(wlo, NWS - 1)
        while cur < hi:
            seg_end = min(hi, int(reg_end[wcur]) if wcur < NWS else hi)
            if seg_end <= cur:       # dead tail past LSLOT
                seg_end = hi
            segs.append((wcur, cur, seg_end))
            cur = seg_end
            wcur = min(wcur + 1, NWS - 1)
        for i, (wsx, lo2, hi2) in enumerate(segs):
            mm_block.append(b)
            mm_ws.append(wsx)
            mm_start.append(i == 0)
            mm_stop.append(i == len(segs) - 1)
            mm_lo.append(lo2 - lo)
            mm_hi.append(hi2 - lo)
    n_mm = len(mm_block)

    PV = np.full((N_CORES, n_mm * 128), -1.0, np.float32)
    for m in range(n_mm):
        b, lo2, hi2 = mm_block[m], mm_lo[m], mm_hi[m]
        PV[:, m * 128 + lo2: m * 128 + hi2] = \
            srcl[:, b * 128 + lo2: b * 128 + hi2]

    # phase-2 run lists per group
    NRg, nrt, idx_w, VA = [], [], [], []
    for g in range(NG):
        rls = []
        for c in range(N_CORES):
            parts = []
            for wsx in range(NWS):
                k = int(runs_cell[c, wsx, g])
                if k:
                    base = int(cb[c, wsx, g]) // R
                    parts.append(base + np.arange(k))
            rl = np.concatenate(parts) if parts else np.zeros(0, np.int64)
            rls.append(rl)
        mx = max(len(r) for r in rls)
        nr = ((mx + 127) // 128) * 128
        NRg.append(nr)
        nrt.append(nr // 128)
        iw, va = [], []
        for c in range(N_CORES):
            rl = np.zeros(nr, np.int64)
            rl[: len(rls[c])] = rls[c]
            # wrap for dma_gather: per 512-call columns of reshape(-1,16).T
            cols = []
            for b0 in range(0, nr, 512):
                blk = rl[b0: b0 + 512]
                cols.append(blk.reshape(-1, 16).T)
            iw.append(np.concatenate(cols, axis=1).astype(np.int16))
            a = dA[c][(rl[:, None] * R + np.arange(R))].astype(np.float32)
            if len(rls[c]) < nr:    # padded runs contribute nothing
                a[len(rls[c]):] = -1.0
            va.append(a.reshape(nr // 128, 128, R).transpose(1, 0, 2)
                      .reshape(128, -1).astype(BF16))
        idx_w.append(iw)
        VA.append(va)

    deg = np.bincount(dst, minlength=N).astype(np.float32)
    invdeg = []
    for c in range(N_CORES):
        v = np.ones(NPC, np.float32)
        v[:NPC_RAW] = 1.0 / np.maximum(deg[c * NPC_RAW:(c + 1) * NPC_RAW], 1.0)
        invdeg.append(np.tile(v[None, :], (WIN, 1)))

    return {
        "LSLOT_PAD": LSLOT_PAD, "NBLK": NBLK, "NRUN": NRUN, "n_mm": n_mm,
        "mm_block": mm_block, "mm_ws": mm_ws, "mm_start": mm_start,
        "mm_stop": mm_stop, "NRg": NRg, "nrt": nrt,
        "PV": [np.ascontiguousarray(PV[c]).astype(BF16)[None, :] for c in range(N_CORES)],
        "idx_w": idx_w, "VA": VA, "invdeg": invdeg,
    }


def _build(plan):
    import concourse.bacc as bacc
    import concourse.bass as bass
    import concourse.mybir as mybir
    import concourse.tile as tile
    from contextlib import ExitStack

    dt = mybir.dt
    NBLK, NRUN, n_mm = plan["NBLK"], plan["NRUN"], plan["n_mm"]
    NRg, nrt = plan["NRg"], plan["nrt"]
    mm_block, mm_ws = plan["mm_block"], plan["mm_ws"]
    mm_start, mm_stop = plan["mm_start"], plan["mm_stop"]
    SNRT = sum(nrt)

    nc = bacc.Bacc("TRN2", target_bir_lowering=False)

    x_sl = nc.dram_tensor("x_slice", [NPC, D_IN], dt.float32, kind="ExternalInput")
    pv_d = nc.dram_tensor("pv", [1, n_mm * 128], dt.bfloat16, kind="ExternalInput")
    idx_d = nc.dram_tensor("idx", [128, (sum(NRg)) // 16], dt.int16,
                           kind="ExternalInput")
    va_d = nc.dram_tensor("va", [128, SNRT * R], dt.bfloat16, kind="ExternalInput")
    invdeg_d = nc.dram_tensor("invdeg", [128, NPC], dt.float32, kind="ExternalInput")
    idbf_d = nc.dram_tensor("idbf", [128, 128], dt.bfloat16, kind="ExternalInput")
    ioct_d = nc.dram_tensor("ioct", [128, MMC * 128], dt.bfloat16,
                            kind="ExternalInput")
    io2t_d = nc.dram_tensor("io2t", [128, R * 256], dt.bfloat16,
                            kind="ExternalInput")
    wl_d, wr_d, b_d = [], [], []
    for li, (din, dout) in enumerate([(D_IN, D_HID), (D_HID, D_HID),
                                      (D_HID, D_OUT)]):
        wl_d.append(nc.dram_tensor(f"wl{li}", [din, dout], dt.bfloat16,
                                   kind="ExternalInput"))
        wr_d.append(nc.dram_tensor(f"wr{li}", [din, dout], dt.bfloat16,
                                   kind="ExternalInput"))
        b_d.append(nc.dram_tensor(f"b{li}", [128, 1], dt.float32,
                                  kind="ExternalInput"))

    xsb = nc.dram_tensor("xsb", [NPC, D_IN], dt.bfloat16)
    hsl = [nc.dram_tensor(f"hsl{i}", [NPC, D_IN], dt.bfloat16) for i in range(2)]
    hf = [nc.dram_tensor(f"hf{i}", [HTAB, D_IN], dt.bfloat16) for i in range(3)]
    stag2 = [nc.dram_tensor(f"stag{i}", [NRUN, R * 128], dt.bfloat16)
             for i in range(2)]
    out_d = nc.dram_tensor("out", [NPC, D_OUT], dt.float32, kind="ExternalOutput")

    groups = [list(range(N_CORES))]

    with tile.TileContext(nc) as tc, ExitStack() as ctx:
        per = ctx.enter_context(tc.tile_pool(name="persist", bufs=1))
        xpool = ctx.enter_context(tc.tile_pool(name="xch", bufs=2))
        ppool = ctx.enter_context(tc.tile_pool(name="pstrip", bufs=2))
        cpool = ctx.enter_context(tc.tile_pool(name="chunk", bufs=2))
        gpool = ctx.enter_context(tc.tile_pool(name="g2", bufs=2))
        spool = ctx.enter_context(tc.tile_pool(name="sstrip", bufs=2))
        mpool = ctx.enter_context(tc.tile_pool(name="misc", bufs=2))
        pp_e = ctx.enter_context(tc.tile_pool(name="ps_exp", bufs=2, space="PSUM"))
        pp = ctx.enter_context(tc.tile_pool(name="ps_agg", bufs=2, space="PSUM"))
        pp_d = ctx.enter_context(tc.tile_pool(name="ps_dense", bufs=2, space="PSUM"))
        pp_t = ctx.enter_context(tc.tile_pool(name="ps_tr", bufs=1, space="PSUM"))

        def load_const(dram, shape, dtp, tag):
            t = per.tile(shape, dtp, tag=tag, name=tag)
            nc.sync.dma_start(out=t[:], in_=dram[:])
            return t

        idbf = load_const(idbf_d, [128, 128], dt.bfloat16, "idbf")
        ioct = load_const(ioct_d, [128, MMC * 128], dt.bfloat16, "ioct")
        io2t = load_const(io2t_d, [128, R * 256], dt.bfloat16, "io2t")
        invdeg = load_const(invdeg_d, [128, NPC], dt.float32, "invdeg")
        wl = [load_const(wl_d[i], list(wl_d[i].shape), dt.bfloat16, f"wl{i}")
              for i in range(3)]
        wr = [load_const(wr_d[i], list(wr_d[i].shape), dt.bfloat16, f"wr{i}")
              for i in range(3)]
        bias = [load_const(b_d[i], [128, 1], dt.float32, f"b{i}") for i in range(3)]
        idx_sb = load_const(idx_d, [128, sum(NRg) // 16], dt.int16, "idx")
        va_sb = load_const(va_d, [128, SNRT * R], dt.bfloat16, "va")

        # iotas
        iota_col = per.tile([128, 1], dt.bfloat16, tag="iotac")
        nc.gpsimd.iota(iota_col[:], pattern=[[1, 1]], base=0,
                       channel_multiplier=1, allow_small_or_imprecise_dtypes=True)
        iota_row = per.tile([128, 128], dt.bfloat16, tag="iotar")
        nc.gpsimd.iota(iota_row[:], pattern=[[1, 128]], base=0,
                       channel_multiplier=0, allow_small_or_imprecise_dtypes=True)
        iota2t_d = None
        zbias = per.tile([128, 1], dt.float32, tag="zbias")
        nc.gpsimd.memset(zbias[:], 0.0)

        idf = per.tile([128, 128], dt.float32, tag="idf")
        nc.vector.tensor_copy(out=idf[:], in_=idbf[:])
        hT = [per.tile([128, NPC], dt.bfloat16, tag=f"hT{i}", name=f"hT{i}")
              for i in range(2)]

        # bootstrap: bf16 cast + AllGather + hT[0]
        nc.gpsimd.dma_start(out=xsb[:, :], in_=x_sl[:, :])
        nc.gpsimd.collective_compute(
            "AllGather", mybir.AluOpType.bypass, replica_groups=groups,
            ins=[xsb.ap().opt()], outs=[hf[0].ap().opt()])
        for w in range(W):
            rows = mpool.tile([128, D_IN], dt.bfloat16, tag="rows_in")
            nc.sync.dma_start(out=rows[:], in_=xsb[w * 128:(w + 1) * 128, :])
            tp = pp_t.tile([128, 128], dt.bfloat16, tag="tp")
            nc.tensor.transpose(out=tp[:], in_=rows[:], identity=idbf[:])
            nc.vector.tensor_copy(out=hT[0][:, w * 128:(w + 1) * 128], in_=tp[:])

        def do_layer(li, hf_in, hT_in, hT_out, hsl_out, hf_out, last):
            dout = D_OUT if last else D_HID
            stag = stag2[li % 2]
            # ---- phase 1: expand into staging ----
            def build_pchunk(pc):
                nmm = min(MMC, n_mm - pc * MMC)
                pvch = ppool.tile([128, nmm * 128], dt.bfloat16,
                                  tag="pvch", name="pvch")
                nc.sync.dma_start(
                    out=pvch[:],
                    in_=pv_d[:1, pc * MMC * 128: (pc * MMC + nmm) * 128]
                    .partition_broadcast(128))
                pstrip = ppool.tile([128, nmm * 128], dt.bfloat16,
                                    tag="pstrip", name="pstrip")
                nc.vector.tensor_tensor(
                    out=pstrip[:],
                    in0=ioct[:, :nmm * 128],
                    in1=pvch[:],
                    op=mybir.AluOpType.is_equal)
                return pstrip

            def load_xchunk(xc):
                nw = min(CHW, NWS - xc * CHW)
                xt = xpool.tile([128, nw, 128], dt.bfloat16, tag="xch",
                                name="xch")
                nc.sync.dma_start(
                    out=xt[:, :, :],
                    in_=hf_in[xc * CHW * 128: (xc * CHW + nw) * 128, :]
                    .rearrange("(b p) f -> p b f", p=128))
                return xt

            nxc_tot = -(-NWS // CHW)
            npc_tot = -(-n_mm // MMC)
            xchs = {0: load_xchunk(0)}
            pstrips = {0: build_pchunk(0)}
            cur_xc = 0
            chunkb = None
            ps = None
            for m in range(n_mm):
                b, wsx = mm_block[m], mm_ws[m]
                xc = wsx // CHW
                if xc != cur_xc or m == 0:
                    cur_xc = xc
                    if xc + 1 < nxc_tot and xc + 1 not in xchs:
                        xchs[xc + 1] = load_xchunk(xc + 1)
                    xchs.pop(xc - 1, None)
                xch = xchs[xc]
                pc = m // MMC
                if m % MMC == 0 and pc + 1 < npc_tot:
                    pstrips[pc + 1] = build_pchunk(pc + 1)
                    pstrips.pop(pc - 1, None)
                pstrip = pstrips[pc]
                if mm_start[m]:
                    ps = pp_e.tile([128, 128], dt.float32, tag="exp")
                nc.tensor.matmul(
                    out=ps[:],
                    lhsT=pstrip[:, (m - pc * MMC) * 128:(m - pc * MMC + 1) * 128],
                    rhs=xch[:, wsx - xc * CHW, :],
                    start=mm_start[m], stop=mm_stop[m])
                if mm_stop[m]:
                    bb = b % CHB
                    if bb == 0:
                        nb = min(CHB, NBLK - b)
                        chunkb = cpool.tile([128, nb, 128], dt.bfloat16,
                                            tag="chunk", name="chunk")
                    nc.scalar.activation(
                        out=chunkb[:, bb, :], in_=ps[:],
                        func=mybir.ActivationFunctionType.Identity)
                    if bb == nb - 1 or b == NBLK - 1:
                        b0 = b - bb
                        nc.sync.dma_start(
                            out=stag[b0 * 16:(b0 + bb + 1) * 16, :]
                            .rearrange("(b ph) (pl f) -> (ph pl) b f",
                                       ph=16, pl=8, f=128),
                            in_=chunkb[:, :bb + 1, :])
            # ---- phase 2: collect + aggregate + dense ----
            ixo = 0
            vo = 0
            for g in range(NG):
                nr = NRg[g]
                nt = nrt[g]
                g2 = gpool.tile([128, nt, R * 128], dt.bfloat16, tag="g2",
                                name="g2")
                for k0 in range(0, nr, 512):
                    kn = min(512, nr - k0)
                    nc.gpsimd.dma_gather(
                        g2[:, k0 // 128: (k0 + kn) // 128, :],
                        stag[:, :],
                        idx_sb[:, ixo + k0 // 16: ixo + (k0 + kn) // 16],
                        kn, kn, R * 128)
                ixo += nr // 16
                nw = min(2 * 128, (W - 2 * g) * 128)
                pwin = slice(2 * g * 128, 2 * g * 128 + nw)
                sstrip = spool.tile([128, nt * R, 256], dt.bfloat16,
                                    tag="sstrip", name="sstrip")
                for t in range(nt):
                    nc.vector.tensor_tensor(
                        out=sstrip[:, t * R:(t + 1) * R, :],
                        in0=va_sb[:, vo + t * R: vo + (t + 1) * R]
                        .rearrange("p (j o) -> p j o", o=1)
                        .to_broadcast([128, R, 256]),
                        in1=io2t[:, :].rearrange("p (j c) -> p j c", c=256),
                        op=mybir.AluOpType.is_equal)
                ps2 = pp.tile([128, 256], dt.float32, tag="agg")
                nmm2 = nt * R
                for t in range(nt):
                    for pos in range(R):
                        j = t * R + pos
                        nc.tensor.matmul(
                            out=ps2[:, :nw],
                            lhsT=g2[:, t, pos * 128:(pos + 1) * 128],
                            rhs=sstrip[:, j, :nw],
                            start=(j == 0), stop=(j == nmm2 - 1))
                aggsc = mpool.tile([128, 256], dt.bfloat16, tag="aggsc")
                nc.vector.tensor_tensor(out=aggsc[:, :nw], in0=ps2[:, :nw],
                                        in1=invdeg[:, pwin],
                                        op=mybir.AluOpType.mult)
                pd = pp_d.tile([128, 256], dt.float32, tag="dense")
                nc.tensor.matmul(out=pd[:dout, :nw], lhsT=wl[li][:],
                                 rhs=aggsc[:, :nw], start=True, stop=False)
                nc.tensor.matmul(out=pd[:dout, :nw], lhsT=wr[li][:],
                                 rhs=hT_in[:, pwin], start=False, stop=True)
                if not last:
                    nc.scalar.activation(
                        out=hT_out[:, pwin], in_=pd[:, :nw],
                        func=mybir.ActivationFunctionType.Relu,
                        bias=bias[li][:, :1])
                    for hh in range(nw // 128):
                        w = 2 * g + hh
                        win = slice(w * 128, (w + 1) * 128)
                        tp = pp_t.tile([128, 128], dt.bfloat16, tag="tp")
                        nc.tensor.transpose(out=tp[:], in_=hT_out[:, win],
                                            identity=idbf[:])
                        rows = mpool.tile([128, D_IN], dt.bfloat16,
                                          tag="rows_out")
                        nc.vector.tensor_copy(out=rows[:], in_=tp[:])
                        nc.sync.dma_start(out=hsl_out[win, :], in_=rows[:])
                else:
                    oT = mpool.tile([128, 256], dt.float32, tag="oT")
                    nc.scalar.activation(
                        out=oT[:dout, :nw], in_=pd[:dout, :nw],
                        func=mybir.ActivationFunctionType.Identity,
                        bias=bias[li][:dout, :1])
                    if dout < 128:
                        nc.vector.memset(oT[dout:, :nw], 0.0)
                    for hh in range(nw // 128):
                        w = 2 * g + hh
                        win = slice(w * 128, (w + 1) * 128)
                        tp = pp_t.tile([128, 128], dt.float32, tag="tpf")
                        nc.tensor.transpose(out=tp[:], in_=oT[:, hh * 128:(hh + 1) * 128],
                                            identity=idf[:])
                        negmax = mpool.tile([128, 1], dt.float32, tag="negmax")
                        nc.vector.tensor_reduce(out=negmax[:], in_=tp[:, :D_OUT],
                                                axis=mybir.AxisListType.X,
                                                op=mybir.AluOpType.max, negate=True)
                        esb = mpool.tile([128, D_OUT], dt.float32, tag="esb")
                        nc.scalar.activation(out=esb[:], in_=tp[:, :D_OUT],
                                             func=mybir.ActivationFunctionType.Exp,
                                             bias=negmax[:, :1])
                        ssum = mpool.tile([128, 1], dt.float32, tag="ssum")
                        nc.vector.tensor_reduce(out=ssum[:], in_=esb[:],
                                                axis=mybir.AxisListType.X,
                                                op=mybir.AluOpType.add)
                        lns = mpool.tile([128, 1], dt.float32, tag="lns")
                        nc.scalar.activation(out=lns[:], in_=ssum[:],
                                             func=mybir.ActivationFunctionType.Ln)
                        csub = mpool.tile([128, 1], dt.float32, tag="csub")
                        nc.vector.tensor_tensor(out=csub[:], in0=lns[:], in1=negmax[:],
                                                op=mybir.AluOpType.subtract)
                        res = mpool.tile([128, D_OUT], dt.float32, tag="res")
                        nc.vector.tensor_tensor(out=res[:], in0=tp[:, :D_OUT],
                                                in1=csub[:, :1].to_broadcast([128, D_OUT]),
                                                op=mybir.AluOpType.subtract)
                        nc.sync.dma_start(out=out_d[win, :], in_=res[:])
                vo += nt * R
            if not last:
                nc.gpsimd.collective_compute(
                    "AllGather", mybir.AluOpType.bypass, replica_groups=groups,
                    ins=[hsl_out.ap().opt()], outs=[hf_out.ap().opt()])

        do_layer(0, hf[0], hT[0], hT[1], hsl[0], hf[1], last=False)
        do_layer(1, hf[1], hT[1], hT[0], hsl[1], hf[2], last=False)
        do_layer(2, hf[2], hT[0], None, None, None, last=True)

    nc.compile()
    return nc


def _make_inputs(plan, x, W1_l, W1_r, b1, Wm_l, Wm_r, bm, W2_l, W2_r, b2):
    ident = np.eye(128, dtype=np.float32).astype(BF16)

    def pad_bias(b):
        v = np.zeros((128, 1), np.float32)
        v[: len(b), 0] = np.asarray(b, np.float32)
        return v

    common = {
        "idbf": ident,
        "ioct": np.tile(np.arange(128, dtype=np.float32)[:, None],
                        (1, MMC * 128)).astype(BF16),
        "io2t": np.tile(np.tile(np.arange(256, dtype=np.float32), R)[None, :],
                        (128, 1)).astype(BF16),
        "wl0": np.asarray(W1_l, np.float32).astype(BF16),
        "wr0": np.asarray(W1_r, np.float32).astype(BF16),
        "b0": pad_bias(b1),
        "wl1": np.asarray(Wm_l, np.float32).astype(BF16),
        "wr1": np.asarray(Wm_r, np.float32).astype(BF16),
        "b1": pad_bias(bm),
        "wl2": np.asarray(W2_l, np.float32).astype(BF16),
        "wr2": np.asarray(W2_r, np.float32).astype(BF16),
        "b2": pad_bias(b2),
    }
    x = np.asarray(x, np.float32)
    in_maps = []
    for c in range(N_CORES):
        xs = np.zeros((NPC, D_IN), np.float32)
        xs[:NPC_RAW] = x[c * NPC_RAW:(c + 1) * NPC_RAW]
        m = dict(common)
        m["x_slice"] = xs
        m["invdeg"] = plan["invdeg"][c]
        m["pv"] = plan["PV"][c]
        m["idx"] = np.tile(np.concatenate(
            [plan["idx_w"][g][c] for g in range(NG)], axis=1), (8, 1))
        m["va"] = np.concatenate([plan["VA"][g][c] for g in range(NG)], axis=1)
        in_maps.append(m)
    return in_maps


def _postprocess(results):
    out = np.empty((N, D_OUT), np.float32)
    for c in range(N_CORES):
        out[c * NPC_RAW:(c + 1) * NPC_RAW] = results[c]["out"][:NPC_RAW]
    return out


_CACHE = {}


def kernel(x, edge_index, W1_l, W1_r, b1, Wm_l, Wm_r, bm, W2_l, W2_r, b2,
           _trace=False):
    from concourse.bass_utils import run_bass_kernel_spmd

    edge_index = np.asarray(edge_index)
    key = hash(edge_index.tobytes())
    if key not in _CACHE:
        plan = _plan(edge_index)
        nc = _build(plan)
        _CACHE[key] = (plan, nc)
    plan, nc = _CACHE[key]
    in_maps = _make_inputs(plan, x, W1_l, W1_r, b1, Wm_l, Wm_r, bm,
                           W2_l, W2_r, b2)
    res = run_bass_kernel_spmd(nc, in_maps, core_ids=list(range(N_CORES)),
                               trace=_trace)
    out = _postprocess(res.results)
    if _trace:
        kernel._last_exec_ns = res.exec_time_ns
        kernel._last_res = res
    return out
